# revision 1
# baseline (speedup 1.0000x reference)
"""Multi-head attention + residual + LayerNorm on 8 Trainium2 NeuronCores.

Sharding: core c in 0..7 handles batch b = c//4 and query-row quarter
r = c%4 (rows 512r..512r+512 of S=2048), with ALL 16 heads.  key/value
are replicated to every core (host-side staging); each core computes the
full-sequence K^T and V projections itself — measured collectives on this
stack cost ~130us per 2MB AllGather, far more than the ~70us of redundant
PE work, and the local pipeline keeps the PE clock warm.

Per core:
  - transpose x tiles on PE (fp32r, via identity), project:
      Q^T [1024, 512] (own rows),  K^T [1024, 2048] -> local DRAM,
      V [2048, 8, 130] pair-blocks with ones columns -> local DRAM
  - per head pair p, per sk chunk c: S^T = K_h Q_h^T  (PSUM) -> exp
    (ACT, scale 1/8) -> U^T accumulation with lhsT = V_aug; the ones
    column makes row 64 the softmax denominators
  - normalize: reciprocal of sums row, PE K=1 broadcast to 64
    partitions, multiply -> ctx^T [128, 8, 512]
  - out = ctx @ Wo + bo + residual -> LayerNorm -> y rows [512, 1024]

All matmuls in float32r (full-rate fp32 PE path, ~2e-4 rel err).
"""

import sys

if "/opt/trn_rl_repo" not in sys.path:
    sys.path.insert(0, "/opt/trn_rl_repo")

import numpy as np

import concourse.bacc as bacc
import concourse.bass as bass
import concourse.mybir as mybir
import concourse.tile as tile
from concourse.bass import ds, ts
from concourse.bass_utils import run_bass_kernel_spmd

FP32R = mybir.dt.float32r
FP32 = mybir.dt.float32
AF = mybir.ActivationFunctionType
ALU = mybir.AluOpType

N_CORES = 8
B = 2
S = 2048
D = 1024
H = 16
DK = 64
P = 128

SL = S // 4  # 512 local query rows per core
KC = D // P  # 8 contraction chunks over d_model
SQ = SL // P  # 4 sq subchunks of 128 (per 512-row block)
CH = S // P  # 16 sk chunks
PAIRS = H // 2  # 8 head pairs
NB = 4  # 512-row blocks of the full sequence
EPS = 1e-5

_NC_CACHE = {}


def build_nc():
    nc = bacc.Bacc(num_devices=N_CORES)

    xq_d = nc.dram_tensor("xq", [SL, D], FP32R, kind="ExternalInput")
    xk_d = nc.dram_tensor("xk", [S, D], FP32R, kind="ExternalInput")
    xv_d = nc.dram_tensor("xv", [S, D], FP32R, kind="ExternalInput")
    wq_d = nc.dram_tensor("wq", [D, D], FP32R, kind="ExternalInput")
    wk_d = nc.dram_tensor("wk", [D, D], FP32R, kind="ExternalInput")
    wv_d = nc.dram_tensor("wv", [D, D], FP32R, kind="ExternalInput")
    wo_d = nc.dram_tensor("wo", [D, D], FP32R, kind="ExternalInput")
    bq_d = nc.dram_tensor("bq", [D], FP32, kind="ExternalInput")
    bk_d = nc.dram_tensor("bk", [D], FP32, kind="ExternalInput")
    bv_d = nc.dram_tensor("bv", [D], FP32, kind="ExternalInput")
    bo_d = nc.dram_tensor("bo", [D], FP32, kind="ExternalInput")
    gam_d = nc.dram_tensor("gam", [D], FP32, kind="ExternalInput")
    bet_d = nc.dram_tensor("bet", [D], FP32, kind="ExternalInput")
    ident_d = nc.dram_tensor("ident", [P, P], FP32R, kind="ExternalInput")
    ones_d = nc.dram_tensor("ones", [P, 64], FP32R, kind="ExternalInput")

    y_d = nc.dram_tensor("y", [SL, D], FP32, kind="ExternalOutput")

    # local DRAM for the full-sequence K^T and augmented V
    kt_d = nc.dram_tensor("ktf", [D, S], FP32R)
    vf_d = nc.dram_tensor("vf", [S, PAIRS, 130], FP32R)

    with tile.TileContext(nc) as tc:
        with (
            tc.tile_pool(name="consts", bufs=1) as consts,
            tc.tile_pool(name="big", bufs=1) as big,
            tc.tile_pool(name="xtp", bufs=1) as xtp,
            tc.tile_pool(name="wide", bufs=1) as wide,
            tc.tile_pool(name="xnp", bufs=4) as xnp,
            tc.tile_pool(name="wpool", bufs=1) as wpool,
            tc.tile_pool(name="stream", bufs=3) as stream,
            tc.tile_pool(name="kttp", bufs=7) as kttp,
            tc.tile_pool(name="etp", bufs=4) as etp,
            tc.tile_pool(name="vat", bufs=2) as vatp,
            tc.tile_pool(name="small", bufs=2) as small,
            tc.tile_pool(name="psA", bufs=3, space="PSUM") as psA,
            tc.tile_pool(name="psAcc", bufs=2, space="PSUM") as psAcc,
            tc.tile_pool(name="psB", bufs=1, space="PSUM") as psB,
        ):
            # ---- constants ----
            ident = consts.tile([P, P], FP32R)
            nc.sync.dma_start(ident[:], ident_d[:])
            ones64 = consts.tile([P, 64], FP32R)
            nc.sync.dma_start(ones64[:], ones_d[:])
            bq_sb = consts.tile([P, KC], FP32)
            nc.sync.dma_start(bq_sb[:], bq_d.rearrange("(m q) -> q m", q=P))
            bk_sb = consts.tile([P, KC], FP32)
            nc.sync.dma_start(bk_sb[:], bk_d.rearrange("(m q) -> q m", q=P))

            def bcast_load(src, tag):
                t = consts.tile([P, D], FP32, tag=tag)
                ap = bass.AP(tensor=src, offset=0, ap=[[0, P], [1, D]])
                nc.gpsimd.dma_start(out=t[:], in_=ap)
                return t

            bv_b = bcast_load(bv_d, "bv_b")
            bo_b = bcast_load(bo_d, "bo_b")
            gam_b = bcast_load(gam_d, "gam_b")
            bet_b = bcast_load(bet_d, "bet_b")
            eps_t = consts.tile([P, 1], FP32)
            nc.vector.memset(eps_t[:], EPS)

            def load_xT(x_d, row0):
                """x rows [row0:row0+512] -> x^T SBUF [128, KC, 512]."""
                xT = xtp.tile([P, KC, SL], FP32R, tag="xT")
                for i in range(SQ):
                    xt = xnp.tile([P, D], FP32R, tag="xnat")
                    nc.sync.dma_start(xt[:], x_d[ds(row0 + i * P, P), :])
                    for j in range(KC):
                        pt = psA.tile([P, P], FP32R, tag="mm")
                        nc.tensor.transpose(pt[:], xt[:, ts(j, P)], ident[:])
                        nc.vector.tensor_copy(xT[:, j, ts(i, P)], pt[:])
                return xT

            # ---- K^T full sequence -> kt_d, block by block ----
            kt_dr = kt_d.rearrange("(m q) s -> q m s", q=P)
            wk_sb = wpool.tile([P, KC, D], FP32R, tag="wrhs")
            for k in range(KC):
                nc.sync.dma_start(wk_sb[:, k, :], wk_d[ts(k, P), :])
            for blk in range(NB):
                xkT = load_xT(xk_d, blk * SL)
                for m in range(KC):
                    pp = psA.tile([P, SL], FP32, tag="mm")
                    for k in range(KC):
                        nc.tensor.matmul(
                            pp[:],
                            wk_sb[:, k, ts(m, P)],
                            xkT[:, k, :],
                            start=(k == 0),
                            stop=(k == KC - 1),
                        )
                    kev = stream.tile([P, SL], FP32R, tag="kev")
                    nc.scalar.activation(
                        out=kev[:],
                        in_=pp[:],
                        func=AF.Identity,
                        bias=bk_sb[:, m : m + 1],
                    )
                    nc.sync.dma_start(kt_dr[:, m, ds(blk * SL, SL)], kev[:])

            # ---- V full sequence -> vf_d (pair-augmented layout) ----
            vf_dr = vf_d.rearrange("(i q) p e -> q i p e", q=P)
            wv_sb = wpool.tile([P, KC, D], FP32R, tag="wrhs")
            for k in range(KC):
                nc.sync.dma_start(wv_sb[:, k, :], wv_d[ts(k, P), :])
            for blk in range(NB):
                xvT = load_xT(xv_d, blk * SL)
                for n in range(2):
                    for i in range(SQ):
                        pp = psA.tile([P, 512], FP32, tag="mm")
                        for k in range(KC):
                            nc.tensor.matmul(
                                pp[:],
                                xvT[:, k, ts(i, P)],
                                wv_sb[:, k, ds(n * 512, 512)],
                                start=(k == 0),
                                stop=(k == KC - 1),
                            )
                        # vtmp holds [4 pairs x (V_even |1| V_odd |1)] = 520 cols
                        vtmp = stream.tile([P, 4, 130], FP32R, tag="vtmp")
                        vdst = vtmp[:].rearrange("q pl (j e) -> q pl j e", e=65)
                        nc.vector.tensor_tensor(
                            vdst[:, :, :, 0:64],
                            pp[:].rearrange("q (pl j e) -> q pl j e", pl=4, j=2),
                            bv_b[:, ds(n * 512, 512)].rearrange(
                                "q (pl j e) -> q pl j e", pl=4, j=2
                            ),
                            ALU.add,
                        )
                        nc.vector.tensor_copy(
                            vdst[:, :, :, 64:65], ones64[:, 0:8, None].rearrange(
                                "q (pl j) o -> q pl j o", pl=4
                            )
                        )
                        ii = blk * SQ + i
                        nc.sync.dma_start(vf_dr[:, ii, ds(n * 4, 4), :], vtmp[:])

            # ---- Q^T (own rows) ----
            xqT = load_xT(xq_d, 0)
            qt_sb = big.tile([P, KC, SL], FP32R, tag="qt")
            wq_sb = wpool.tile([P, KC, D], FP32R, tag="wrhs")
            for k in range(KC):
                nc.sync.dma_start(wq_sb[:, k, :], wq_d[ts(k, P), :])
            for m in range(KC):
                pp = psA.tile([P, SL], FP32, tag="mm")
                for k in range(KC):
                    nc.tensor.matmul(
                        pp[:],
                        wq_sb[:, k, ts(m, P)],
                        xqT[:, k, :],
                        start=(k == 0),
                        stop=(k == KC - 1),
                    )
                nc.scalar.activation(
                    out=qt_sb[:, m, :],
                    in_=pp[:],
                    func=AF.Identity,
                    bias=bq_sb[:, m : m + 1],
                )

            # ---- attention ----
            ctx_sb = big.tile([P, PAIRS, SL], FP32R, tag="ctx")
            vf_blk = vf_d.rearrange("(b i q) p e -> q b i p e", b=NB, q=P)

            def emit_normalize(np_, uA, uB):
                # rows 0..63 of ut / row 64 -> ctx_sb[:, np_, :]
                for j, ut in enumerate((uA, uB)):
                    rec = small.tile([P, SL], FP32R, tag="rec")
                    with nc.allow_low_precision(
                        reason="float32r is bit-identical to float32 in SBUF"
                    ):
                        nc.vector.reciprocal(out=rec[64:65, :], in_=ut[64:65, :])
                    bc = psB.tile([P, SL], FP32, tag="bc")
                    nc.tensor.matmul(
                        bc[0:64, :],
                        ones64[64:65, :],
                        rec[64:65, :],
                        start=True,
                        stop=True,
                    )
                    bc_sb = small.tile([P, SL], FP32, tag="bcs")
                    nc.vector.tensor_copy(bc_sb[0:64, :], bc[0:64, :])
                    if j == 0:
                        nc.vector.tensor_tensor(
                            ctx_sb[0:64, np_, :], ut[0:64, :], bc_sb[0:64, :], ALU.mult
                        )
                    else:
                        ctmp = small.tile([P, SL], FP32R, tag="ctmp")
                        nc.vector.tensor_tensor(
                            ctmp[0:64, :], ut[0:64, :], bc_sb[0:64, :], ALU.mult
                        )
                        # partition shift 0-63 -> 64-127 via SBUF-SBUF DMA
                        nc.sync.dma_start(ctx_sb[64:128, np_, :], ctmp[0:64, :])

            def prefetch_pair(pp_):
                vt = vatp.tile([P, NB, SQ, 130], FP32R, tag="vat", name=f"vt_{pp_}")
                for b in range(NB):
                    nc.sync.dma_start(vt[:, b], vf_blk[:, b, :, pp_, :])
                ktts = []
                for sb4 in range(NB):
                    kq = kttp.tile(
                        [P, SL], FP32R, tag="ktt", name=f"ktt_{pp_}_{sb4}"
                    )
                    nc.sync.dma_start(kq[:], kt_dr[:, pp_, ds(sb4 * SL, SL)])
                    ktts.append(kq)
                return vt, ktts

            tiles = {0: prefetch_pair(0)}
            norm_pend = None
            for p in range(PAIRS):
                utA = psAcc.tile([P, SL], FP32, tag="accA")
                utB = psAcc.tile([P, SL], FP32, tag="accB")
                vt, ktts = tiles.pop(p)
                # software pipeline: issue S^T/exp for chunk c+1 before the
                # U^T matmuls of chunk c, so the in-order PE never stalls on
                # ACT; the previous pair's normalize is likewise deferred into
                # this pair's stream so its PE broadcast never waits on DVE.
                pend = None
                for c in range(CH):
                    ktt = ktts[c // SQ][:, ts(c % SQ, P)]
                    ets = []
                    for j in range(2):
                        st = psA.tile([P, SL], FP32, tag="mm")
                        nc.tensor.matmul(
                            st[:],
                            ktt[ds(j * 64, 64), :],
                            qt_sb[ds(j * 64, 64), p, :],
                            start=True,
                            stop=True,
                        )
                        et = etp.tile([P, SL], FP32R, tag="et")
                        nc.scalar.activation(
                            out=et[:], in_=st[:], func=AF.Exp, scale=0.125
                        )
                        ets.append(et)
                    if c == 7 and norm_pend is not None:
                        emit_normalize(*norm_pend)
                        norm_pend = None
                    if c == 4 and p + 1 < PAIRS:
                        tiles[p + 1] = prefetch_pair(p + 1)
                    if pend is not None:
                        pc, pets, pv = pend
                        for j, ut in enumerate((utA, utB)):
                            nc.tensor.matmul(
                                ut[:65, :],
                                pv[:, ds(j * 65, 65)],
                                pets[j][:],
                                start=(pc == 0),
                                stop=False,
                            )
                    pend = (c, ets, vt[:, c // SQ, c % SQ, :])
                pc, pets, pv = pend
                for j, ut in enumerate((utA, utB)):
                    nc.tensor.matmul(
                        ut[:65, :],
                        pv[:, ds(j * 65, 65)],
                        pets[j][:],
                        start=False,
                        stop=True,
                    )
                norm_pend = (p, utA, utB)
            emit_normalize(*norm_pend)

            # ---- output projection + residual + LayerNorm ----
            out_sb = big.tile([P, SQ, D], FP32, tag="out")
            wo_sb = wpool.tile([P, KC, D], FP32R, tag="wrhs")
            for k in range(KC):
                nc.sync.dma_start(wo_sb[:, k, :], wo_d[ts(k, P), :])
            # i-outer so each row chunk's LayerNorm starts as soon as its
            # two 512-col halves are projected, instead of after all of them
            for i in range(SQ):
                for n in range(2):
                    pp = psA.tile([P, 512], FP32, tag="mm")
                    for p in range(PAIRS):
                        nc.tensor.matmul(
                            pp[:],
                            ctx_sb[:, p, ts(i, P)],
                            wo_sb[:, p, ds(n * 512, 512)],
                            start=(p == 0),
                            stop=(p == PAIRS - 1),
                        )
                    res = stream.tile([P, 512], FP32R, tag="res")
                    nc.sync.dma_start(res[:], xq_d[ts(i, P), ds(n * 512, 512)])
                    tmp = stream.tile([P, 512], FP32, tag="otmp")
                    nc.vector.tensor_tensor(tmp[:], pp[:], res[:], ALU.add)
                    nc.vector.tensor_tensor(
                        out_sb[:, i, ds(n * 512, 512)],
                        tmp[:],
                        bo_b[:, ds(n * 512, 512)],
                        ALU.add,
                    )
                row = out_sb[:, i, :]
                stats = small.tile([P, 2, 6], FP32, tag="stats")
                nc.vector.bn_stats(stats[:, 0, :], row[:, 0:512])
                nc.vector.bn_stats(stats[:, 1, :], row[:, 512:1024])
                mv = small.tile([P, 2], FP32, tag="mv")
                nc.vector.bn_aggr(mv[:], stats[:])
                std = small.tile([P, 1], FP32, tag="std")
                nc.scalar.activation(
                    out=std[:], in_=mv[:, 1:2], func=AF.Sqrt, bias=eps_t[:], scale=1.0
                )
                rstd = small.tile([P, 1], FP32, tag="rstd")
                nc.vector.reciprocal(out=rstd[:], in_=std[:])
                ytile = wide.tile([P, D], FP32, tag="y")
                nc.vector.tensor_scalar(
                    out=ytile[:],
                    in0=row,
                    scalar1=mv[:, 0:1],
                    scalar2=rstd[:],
                    op0=ALU.subtract,
                    op1=ALU.mult,
                )
                nc.vector.tensor_tensor(ytile[:], ytile[:], gam_b[:], ALU.mult)
                nc.vector.tensor_tensor(ytile[:], ytile[:], bet_b[:], ALU.add)
                nc.sync.dma_start(y_d[ts(i, P), :], ytile[:])

    nc.compile()
    return nc


def get_nc():
    if "nc" not in _NC_CACHE:
        _NC_CACHE["nc"] = build_nc()
    return _NC_CACHE["nc"]


def kernel(
    query,
    key,
    value,
    Wq,
    bq,
    Wk,
    bk,
    Wv,
    bv,
    Wo,
    bo,
    ln_gamma,
    ln_beta,
    _trace=False,
    _trace_cores=None,
):
    query = np.ascontiguousarray(np.asarray(query, dtype=np.float32))
    key = np.ascontiguousarray(np.asarray(key, dtype=np.float32))
    value = np.ascontiguousarray(np.asarray(value, dtype=np.float32))
    shared = {
        "wq": np.ascontiguousarray(np.asarray(Wq, np.float32)),
        "wk": np.ascontiguousarray(np.asarray(Wk, np.float32)),
        "wv": np.ascontiguousarray(np.asarray(Wv, np.float32)),
        "wo": np.ascontiguousarray(np.asarray(Wo, np.float32)),
        "bq": np.ascontiguousarray(np.asarray(bq, np.float32)),
        "bk": np.ascontiguousarray(np.asarray(bk, np.float32)),
        "bv": np.ascontiguousarray(np.asarray(bv, np.float32)),
        "bo": np.ascontiguousarray(np.asarray(bo, np.float32)),
        "gam": np.ascontiguousarray(np.asarray(ln_gamma, np.float32)),
        "bet": np.ascontiguousarray(np.asarray(ln_beta, np.float32)),
        "ident": np.eye(P, dtype=np.float32),
        "ones": np.ones((P, 64), dtype=np.float32),
    }
    in_maps = []
    for c in range(N_CORES):
        b, r = divmod(c, NB)
        rows = slice(r * SL, (r + 1) * SL)
        m = dict(shared)
        m["xq"] = np.ascontiguousarray(query[b, rows, :])
        m["xk"] = np.ascontiguousarray(key[b])
        m["xv"] = np.ascontiguousarray(value[b])
        in_maps.append(m)

    nc = get_nc()
    res = run_bass_kernel_spmd(
        nc,
        in_maps,
        list(range(N_CORES)),
        trace=_trace,
        trace_cores=_trace_cores,
    )
    out = np.empty((B, S, D), dtype=np.float32)
    for c in range(N_CORES):
        b, r = divmod(c, NB)
        out[b, r * SL : (r + 1) * SL, :] = res.results[c]["y"]
    if _trace:
        return out, res
    return out



# revision 2
# speedup vs baseline: 1.3272x; 1.3272x over previous
"""Multi-head attention + residual + LayerNorm on 8 Trainium2 NeuronCores.

Sharding: core c in 0..7 handles batch b = c//4 and query-row quarter
r = c%4 (rows 512r..512r+512 of S=2048), with ALL 16 heads.  key/value
are replicated per batch (host-side staging); each core computes the
full-sequence K^T and V projections itself — measured collectives on
this stack cost ~130us per 2MB AllGather, far more than the redundant
PE work, and the local pipeline keeps the PE clock warm.

v2 changes vs the 543us baseline (which was PE-bound with a 1.64x
stretch from serialized fp32 LDWEIGHTS + HAM cold-clock, plus a 25us
dead preamble and DRAM roundtrips for K^T/V):
  - host stages x^T (pre-transposed) and all matmul operands in bf16:
    no PE transposes, FWL-fast weight loads, half the DMA bytes
  - K^T, V_aug, Q^T, ctx all SBUF-resident (no DRAM roundtrip, no
    per-pair prefetch stalls -> PE stays HAM-warm)
  - one weight load feeds 2 matmuls in the projection loops
  - ~28 warmup matmuls + dummy exp during the DMA preamble warm the
    PE clock and preload the ACT exp table
  - psum: tags mm(3) + ut(4) + bc(1) = 8 banks; softmax normalize of
    pair p-1 is emitted inside pair p so nothing blocks
  - LayerNorm normalization via one ACT op (per-partition scale/bias),
    residual+bo folded on host

Accumulations stay fp32 in PSUM; softmax reciprocal / LN stats fp32.
"""

import sys

if "/opt/trn_rl_repo" not in sys.path:
    sys.path.insert(0, "/opt/trn_rl_repo")

import ml_dtypes
import numpy as np

import concourse.bacc as bacc
import concourse.bass as bass
import concourse.mybir as mybir
import concourse.tile as tile
from concourse.bass import ds, ts
from concourse.bass_utils import run_bass_kernel_spmd

FP32R = mybir.dt.float32r
FP32 = mybir.dt.float32
BF16 = mybir.dt.bfloat16
AF = mybir.ActivationFunctionType
ALU = mybir.AluOpType
BFNP = ml_dtypes.bfloat16

N_CORES = 8
B = 2
S = 2048
D = 1024
H = 16
DK = 64
P = 128

SL = S // 4  # 512 local query rows per core
KC = D // P  # 8 contraction chunks over d_model
SQ = SL // P  # 4 sq subchunks of 128 (per 512-row block)
CH = S // P  # 16 sk chunks
PAIRS = H // 2  # 8 head pairs
NB = 4  # row quarters
EPS = 1e-5

_NC_CACHE = {}


def build_nc():
    nc = bacc.Bacc(num_devices=N_CORES)

    xqt_d = nc.dram_tensor("xqt", [D, SL], BF16, kind="ExternalInput")
    xkt_d = nc.dram_tensor("xkt", [D, S], BF16, kind="ExternalInput")
    xvt_d = nc.dram_tensor("xvt", [D, S], BF16, kind="ExternalInput")
    res_d = nc.dram_tensor("resg", [SL, D], FP32, kind="ExternalInput")
    wq_d = nc.dram_tensor("wq", [D, D], BF16, kind="ExternalInput")
    wk_d = nc.dram_tensor("wk", [D, D], BF16, kind="ExternalInput")
    wv_d = nc.dram_tensor("wv", [D, D], BF16, kind="ExternalInput")
    wo_d = nc.dram_tensor("wo", [D, D], BF16, kind="ExternalInput")
    bq_d = nc.dram_tensor("bq", [D], FP32, kind="ExternalInput")
    bk_d = nc.dram_tensor("bk", [D], FP32, kind="ExternalInput")
    bv_d = nc.dram_tensor("bv", [D], BF16, kind="ExternalInput")
    gam_d = nc.dram_tensor("gam", [D], FP32, kind="ExternalInput")
    bet_d = nc.dram_tensor("bet", [D], FP32, kind="ExternalInput")
    ones_d = nc.dram_tensor("ones", [P, 64], FP32R, kind="ExternalInput")

    y_d = nc.dram_tensor("y", [SL, D], FP32, kind="ExternalOutput")

    wq_r = wq_d.rearrange("(c q) m -> q c m", q=P)
    wk_r = wk_d.rearrange("(c q) m -> q c m", q=P)
    wv_r = wv_d.rearrange("(c q) m -> q c m", q=P)
    wo_r = wo_d.rearrange("(c q) m -> q c m", q=P)
    xqt_r = xqt_d.rearrange("(c q) s -> q c s", q=P)
    xkt_r = xkt_d.rearrange("(c q) s -> q c s", q=P)
    xvt_r = xvt_d.rearrange("(c q) s -> q c s", q=P)

    with tile.TileContext(nc) as tc:
        with (
            tc.tile_pool(name="consts", bufs=1) as consts,
            tc.tile_pool(name="big", bufs=1) as big,
            tc.tile_pool(name="xvp", bufs=3) as xvp,
            tc.tile_pool(name="wcol", bufs=3) as wcol,
            tc.tile_pool(name="wres", bufs=1) as wres,
            tc.tile_pool(name="etp", bufs=4) as etp,
            tc.tile_pool(name="normp", bufs=2) as normp,
            tc.tile_pool(name="outp", bufs=2) as outp,
            tc.tile_pool(name="small", bufs=2) as small,
            tc.tile_pool(name="ps", bufs=1, space="PSUM") as ps,
        ):
            # ---- constants ----
            ones64 = consts.tile([P, 64], FP32R)
            nc.sync.dma_start(ones64[:], ones_d[:])
            bq_sb = consts.tile([P, KC], FP32)
            nc.sync.dma_start(bq_sb[:], bq_d.rearrange("(m q) -> q m", q=P))
            bk_sb = consts.tile([P, KC], FP32)
            nc.sync.dma_start(bk_sb[:], bk_d.rearrange("(m q) -> q m", q=P))

            def bcast_load(src, tag, dt):
                t = consts.tile([P, D], dt, tag=tag)
                ap = bass.AP(tensor=src, offset=0, ap=[[0, P], [1, D]])
                nc.gpsimd.dma_start(out=t[:], in_=ap)
                return t

            bv_b = bcast_load(bv_d, "bv_b", BF16)
            gam_b = bcast_load(gam_d, "gam_b", FP32)
            bet_b = bcast_load(bet_d, "bet_b", FP32)
            eps_t = consts.tile([P, 1], FP32)
            nc.vector.memset(eps_t[:], EPS)

            # ---- PE clock warmup + ACT exp table preload (runs during
            # the input DMA preamble; results are never read) ----
            warm = consts.tile([P, P], BF16)
            nc.vector.memset(warm[:], 0.001)
            wx1 = consts.tile([P, 1], FP32)
            nc.vector.memset(wx1[:], 0.0)
            wxo = consts.tile([P, 1], BF16)
            nc.scalar.activation(out=wxo[:], in_=wx1[:], func=AF.Exp, scale=0.125)
            for _ in range(28):
                pw = ps.tile([P, SL], FP32, tag="mm", bufs=3)
                nc.tensor.matmul(pw[:, 0:P], warm[:], warm[:], start=True, stop=True)

            # ---- Q^T projection (own rows): qt[q, m, s] = Q^T ----
            xqT = big.tile([P, KC, SL], BF16, tag="xqT")
            nc.sync.dma_start(xqT[:], xqt_r[:])
            qt = big.tile([P, KC, SL], BF16, tag="qt")
            for m in range(KC):
                wq_t = wcol.tile([P, KC, P], BF16, tag="wcol")
                nc.sync.dma_start(wq_t[:], wq_r[:, :, ts(m, P)])
                pq = ps.tile([P, SL], FP32, tag="mm", bufs=3)
                for k in range(KC):
                    nc.tensor.matmul(
                        pq[:],
                        wq_t[:, k, :],
                        xqT[:, k, :],
                        start=(k == 0),
                        stop=(k == KC - 1),
                    )
                nc.scalar.activation(
                    out=qt[:, m, :], in_=pq[:], func=AF.Identity,
                    bias=bq_sb[:, m : m + 1],
                )

            # ---- K^T projection (full sequence, SBUF-resident) ----
            xkT = big.tile([P, KC, S], BF16, tag="xkT")
            nc.sync.dma_start(xkT[:], xkt_r[:])
            ktf = big.tile([P, KC, S], BF16, tag="ktf")
            for m in range(KC):
                wk_t = wcol.tile([P, KC, P], BF16, tag="wcol")
                nc.sync.dma_start(wk_t[:], wk_r[:, :, ts(m, P)])
                for g in range(2):
                    pk0 = ps.tile([P, SL], FP32, tag="mm", bufs=3)
                    pk1 = ps.tile([P, SL], FP32, tag="mm", bufs=3)
                    pks = (pk0, pk1)
                    for k in range(KC):
                        for q2 in range(2):
                            sq = g * 2 + q2
                            nc.tensor.matmul(
                                pks[q2][:],
                                wk_t[:, k, :],
                                xkT[:, k, ds(sq * SL, SL)],
                                start=(k == 0),
                                stop=(k == KC - 1),
                            )
                    for q2 in range(2):
                        sq = g * 2 + q2
                        dst = ktf[:, m, ds(sq * SL, SL)]
                        if q2 == 0:
                            nc.scalar.activation(
                                out=dst, in_=pks[q2][:], func=AF.Identity,
                                bias=bk_sb[:, m : m + 1],
                            )
                        else:
                            nc.vector.tensor_scalar_add(
                                dst, pks[q2][:], bk_sb[:, m : m + 1]
                            )

            # ---- V projection (full sequence, pair-augmented, SBUF) ----
            # vf[q, sc, pair, 130] = [V_even 64 | 1 | V_odd 64 | 1] bf16
            vf = big.tile([P, CH, PAIRS, 130], BF16, tag="vf")
            nc.vector.memset(vf[:, :, :, 64:65], 1.0)
            nc.vector.memset(vf[:, :, :, 129:130], 1.0)
            wv_sb = wres.tile([P, KC, D], BF16, tag="wres")
            nc.sync.dma_start(wv_sb[:], wv_r[:])
            for sc in range(CH):
                xv_t = xvp.tile([P, KC, P], BF16, tag="xv")
                nc.sync.dma_start(xv_t[:], xvt_r[:, :, ts(sc, P)])
                pv0 = ps.tile([P, 512], FP32, tag="mm", bufs=3)
                pv1 = ps.tile([P, 512], FP32, tag="mm", bufs=3)
                pvs = (pv0, pv1)
                for k in range(KC):
                    for half in range(2):
                        nc.tensor.matmul(
                            pvs[half][:],
                            xv_t[:, k, :],
                            wv_sb[:, k, ds(half * 512, 512)],
                            start=(k == 0),
                            stop=(k == KC - 1),
                        )
                for half in range(2):
                    vdst = vf[:, sc, ds(half * 4, 4), :].rearrange(
                        "q pl (j e) -> q pl j e", e=65
                    )
                    nc.vector.tensor_tensor(
                        vdst[:, :, :, 0:64],
                        pvs[half][:].rearrange("q (pl j e) -> q pl j e", pl=4, j=2),
                        bv_b[:, ds(half * 512, 512)].rearrange(
                            "q (pl j e) -> q pl j e", pl=4, j=2
                        ),
                        ALU.add,
                    )

            # wo into the wres slot (DMA runs during attention, after the
            # last wv read)
            wo_sb = wres.tile([P, KC, D], BF16, tag="wres")
            nc.sync.dma_start(wo_sb[:], wo_r[:])

            # ---- attention ----
            ctx = big.tile([P, PAIRS, SL], BF16, tag="ctx")

            def emit_normalize(p_, uA, uB):
                # rows 0..63 of ut / row 64 -> ctx[:, p_, :]
                for j, ut in enumerate((uA, uB)):
                    rec = normp.tile([P, SL], FP32R, tag="rec")
                    with nc.allow_low_precision(
                        reason="float32r is bit-identical to float32 in SBUF"
                    ):
                        nc.vector.reciprocal(out=rec[64:65, :], in_=ut[64:65, :])
                    bc = ps.tile([P, SL], FP32, tag="bc", bufs=1)
                    nc.tensor.matmul(
                        bc[0:64, :],
                        ones64[64:65, :],
                        rec[64:65, :],
                        start=True,
                        stop=True,
                    )
                    bc_sb = normp.tile([P, SL], FP32, tag="bcs")
                    nc.vector.tensor_copy(bc_sb[0:64, :], bc[0:64, :])
                    if j == 0:
                        nc.vector.tensor_tensor(
                            ctx[0:64, p_, :], ut[0:64, :], bc_sb[0:64, :], ALU.mult
                        )
                    else:
                        ctmp = normp.tile([P, SL], BF16, tag="ctmp")
                        nc.vector.tensor_tensor(
                            ctmp[0:64, :], ut[0:64, :], bc_sb[0:64, :], ALU.mult
                        )
                        # partition shift 0-63 -> 64-127 via SBUF-SBUF DMA
                        nc.sync.dma_start(ctx[64:128, p_, :], ctmp[0:64, :])

            norm_pend = None
            for p in range(PAIRS):
                utA = ps.tile([P, SL], FP32, tag="ut", bufs=4)
                utB = ps.tile([P, SL], FP32, tag="ut", bufs=4)
                # software pipeline: S^T/exp for chunk c+1 issue before the
                # U^T matmuls of chunk c; pair p-1's normalize is emitted
                # inside pair p so the PE never waits on DVE.
                pend = None
                for c in range(CH):
                    ets = []
                    for j in range(2):
                        st = ps.tile([P, SL], FP32, tag="mm", bufs=3)
                        nc.tensor.matmul(
                            st[:],
                            ktf[ds(j * 64, 64), p, ts(c, P)],
                            qt[ds(j * 64, 64), p, :],
                            start=True,
                            stop=True,
                        )
                        et = etp.tile([P, SL], BF16, tag="et")
                        nc.scalar.activation(
                            out=et[:], in_=st[:], func=AF.Exp, scale=0.125
                        )
                        ets.append(et)
                    if c == 1 and norm_pend is not None:
                        emit_normalize(*norm_pend)
                        norm_pend = None
                    if pend is not None:
                        pc, pets = pend
                        for j, ut in enumerate((utA, utB)):
                            nc.tensor.matmul(
                                ut[:65, :],
                                vf[:, pc, p, ds(j * 65, 65)],
                                pets[j][:],
                                start=(pc == 0),
                                stop=False,
                            )
                    pend = (c, ets)
                pc, pets = pend
                for j, ut in enumerate((utA, utB)):
                    nc.tensor.matmul(
                        ut[:65, :],
                        vf[:, pc, p, ds(j * 65, 65)],
                        pets[j][:],
                        start=False,
                        stop=True,
                    )
                norm_pend = (p, utA, utB)
            emit_normalize(*norm_pend)

            # ---- output projection + residual(+bo) + LayerNorm ----
            for i in range(SQ):
                res_t = outp.tile([P, D], FP32, tag="res")
                nc.sync.dma_start(res_t[:], res_d[ts(i, P), :])
                orow = outp.tile([P, D], FP32, tag="orow")
                for n in range(2):
                    po = ps.tile([P, 512], FP32, tag="mm", bufs=3)
                    for pp_ in range(PAIRS):
                        nc.tensor.matmul(
                            po[:],
                            ctx[:, pp_, ts(i, P)],
                            wo_sb[:, pp_, ds(n * 512, 512)],
                            start=(pp_ == 0),
                            stop=(pp_ == PAIRS - 1),
                        )
                    nc.vector.tensor_tensor(
                        orow[:, ds(n * 512, 512)], po[:],
                        res_t[:, ds(n * 512, 512)], ALU.add,
                    )
                stats = small.tile([P, 2, 6], FP32, tag="stats")
                nc.vector.bn_stats(stats[:, 0, :], orow[:, 0:512])
                nc.vector.bn_stats(stats[:, 1, :], orow[:, 512:1024])
                mv = small.tile([P, 2], FP32, tag="mv")
                nc.vector.bn_aggr(mv[:], stats[:])
                std = small.tile([P, 1], FP32, tag="std")
                nc.scalar.activation(
                    out=std[:], in_=mv[:, 1:2], func=AF.Sqrt, bias=eps_t[:], scale=1.0
                )
                rstd = small.tile([P, 1], FP32, tag="rstd")
                nc.vector.reciprocal(out=rstd[:], in_=std[:])
                nmr = small.tile([P, 1], FP32, tag="nmr")
                nc.vector.tensor_scalar(
                    out=nmr[:], in0=mv[:, 0:1], scalar1=rstd[:], scalar2=-1.0,
                    op0=ALU.mult, op1=ALU.mult,
                )
                yt = outp.tile([P, D], FP32, tag="yt")
                nc.scalar.activation(
                    out=yt[:], in_=orow[:], func=AF.Identity,
                    bias=nmr[:], scale=rstd[:],
                )
                nc.vector.tensor_tensor(yt[:], yt[:], gam_b[:], ALU.mult)
                nc.vector.tensor_tensor(yt[:], yt[:], bet_b[:], ALU.add)
                nc.sync.dma_start(y_d[ts(i, P), :], yt[:])

    nc.compile()
    return nc


def get_nc():
    if "nc" not in _NC_CACHE:
        _NC_CACHE["nc"] = build_nc()
    return _NC_CACHE["nc"]


def kernel(
    query,
    key,
    value,
    Wq,
    bq,
    Wk,
    bk,
    Wv,
    bv,
    Wo,
    bo,
    ln_gamma,
    ln_beta,
    _trace=False,
    _trace_cores=None,
):
    query = np.ascontiguousarray(np.asarray(query, dtype=np.float32))
    key = np.ascontiguousarray(np.asarray(key, dtype=np.float32))
    value = np.ascontiguousarray(np.asarray(value, dtype=np.float32))
    bo_f = np.asarray(bo, np.float32)
    shared = {
        "wq": np.ascontiguousarray(np.asarray(Wq, np.float32).astype(BFNP)),
        "wk": np.ascontiguousarray(np.asarray(Wk, np.float32).astype(BFNP)),
        "wv": np.ascontiguousarray(np.asarray(Wv, np.float32).astype(BFNP)),
        "wo": np.ascontiguousarray(np.asarray(Wo, np.float32).astype(BFNP)),
        "bq": np.ascontiguousarray(np.asarray(bq, np.float32)),
        "bk": np.ascontiguousarray(np.asarray(bk, np.float32)),
        "bv": np.ascontiguousarray(np.asarray(bv, np.float32).astype(BFNP)),
        "gam": np.ascontiguousarray(np.asarray(ln_gamma, np.float32)),
        "bet": np.ascontiguousarray(np.asarray(ln_beta, np.float32)),
        "ones": np.ones((P, 64), dtype=np.float32),
    }
    kT = [np.ascontiguousarray(key[b].T.astype(BFNP)) for b in range(B)]
    vT = [np.ascontiguousarray(value[b].T.astype(BFNP)) for b in range(B)]
    in_maps = []
    for c in range(N_CORES):
        b, r = divmod(c, NB)
        rows = slice(r * SL, (r + 1) * SL)
        xq_rows = query[b, rows, :]
        m = dict(shared)
        m["xqt"] = np.ascontiguousarray(xq_rows.T.astype(BFNP))
        m["xkt"] = kT[b]
        m["xvt"] = vT[b]
        m["resg"] = np.ascontiguousarray(xq_rows + bo_f[None, :])
        in_maps.append(m)

    nc = get_nc()
    res = run_bass_kernel_spmd(
        nc,
        in_maps,
        list(range(N_CORES)),
        trace=_trace,
        trace_cores=_trace_cores,
    )
    out = np.empty((B, S, D), dtype=np.float32)
    for c in range(N_CORES):
        b, r = divmod(c, NB)
        out[b, r * SL : (r + 1) * SL, :] = res.results[c]["y"]
    if _trace:
        return out, res
    return out


# revision 8
# speedup vs baseline: 1.5609x; 1.1761x over previous
"""Multi-head attention + residual + LayerNorm on 8 Trainium2 NeuronCores.

Sharding: core c in 0..7 handles batch b = c//4 and query-row quarter
r = c%4 (rows 512r..512r+512 of S=2048), with ALL 16 heads.  key/value
are replicated per batch (host-side staging); each core computes the
full-sequence K^T and V projections itself — measured collectives on
this stack cost ~130us per 2MB AllGather, far more than the redundant
PE work, and the local pipeline keeps the PE clock warm.

v3 (vs 542us fp32r baseline, 408us v2):
  - host stages x^T (pre-transposed) and all matmul operands in bf16:
    no PE transposes, FWL weight loads, half the DMA bytes
  - K^T, V_aug, Q^T, ctx all SBUF-resident (no DRAM roundtrips)
  - all PSUM matmul tiles are [128, 1024] 2-bank tiles (tag mm2 ring 2
    + softmax accumulators ut ring 4 = 8 banks): projections pair two
    512-col accumulators per tile and evacuate with ONE wide ACT op;
    attention computes both heads' scores into one tile and exps them
    with ONE 1024-wide ACTIVATE (the ACT 352-cycle/instr overhead was
    pacing the attention phase at 1440ns/chunk vs PE's ~1000ns)
  - softmax denominator broadcast via DRAM-bounce DMA (partition-
    stride-0 read) instead of a PE matmul: normalize is entirely off
    the PE critical path, so pairs pipeline without stalls
  - attention software pipeline crosses pair boundaries (U^T matmuls
    of chunk c issue during chunk c+1, last chunk drains into the next
    pair's first chunk)
  - bulk DMAs ride separate engine queues so the Q-projection feed is
    not queued behind the 4MB K/V loads
  - ~36 warmup matmuls + a dummy exp during the DMA preamble warm the
    PE clock (HAM) and preload the ACT exp table
  - LayerNorm: residual+bo folded on host, normalization via one ACT
    op with per-partition scale/bias; gamma/beta applied only when
    they are non-trivial (checked on host, separate compiled variant)

Accumulations stay fp32 in PSUM; softmax reciprocal / LN stats fp32.
"""

import sys

if "/opt/trn_rl_repo" not in sys.path:
    sys.path.insert(0, "/opt/trn_rl_repo")

import ml_dtypes
import numpy as np

import concourse.bacc as bacc
import concourse.bass as bass
import concourse.mybir as mybir
import concourse.tile as tile
from concourse.bass import ds, ts
from concourse.bass_utils import run_bass_kernel_spmd

FP32R = mybir.dt.float32r
FP32 = mybir.dt.float32
BF16 = mybir.dt.bfloat16
AF = mybir.ActivationFunctionType
ALU = mybir.AluOpType
BFNP = ml_dtypes.bfloat16

N_CORES = 8
B = 2
S = 2048
D = 1024
H = 16
DK = 64
P = 128

SL = S // 4  # 512 local query rows per core
KC = D // P  # 8 contraction chunks over d_model
SQ = SL // P  # 4 sq subchunks of 128 (per 512-row block)
CH = S // P  # 16 sk chunks
PAIRS = H // 2  # 8 head pairs
NB = 4  # row quarters
EPS = 1e-5

_NC_CACHE = {}


def build_nc(apply_gb: bool):
    nc = bacc.Bacc(num_devices=N_CORES)

    xqt_d = nc.dram_tensor("xqt", [D, SL], BF16, kind="ExternalInput")
    xkt_d = nc.dram_tensor("xkt", [D, S], BF16, kind="ExternalInput")
    xvt_d = nc.dram_tensor("xvt", [D, S], BF16, kind="ExternalInput")
    res_d = nc.dram_tensor("resg", [SL, D], FP32, kind="ExternalInput")
    wq_d = nc.dram_tensor("wq", [D, D], BF16, kind="ExternalInput")
    wk_d = nc.dram_tensor("wk", [D, D], BF16, kind="ExternalInput")
    wv_d = nc.dram_tensor("wv", [D, D], BF16, kind="ExternalInput")
    wo_d = nc.dram_tensor("wo", [D, D], BF16, kind="ExternalInput")
    bq_d = nc.dram_tensor("bq", [D], FP32, kind="ExternalInput")
    bk_d = nc.dram_tensor("bk", [D], FP32, kind="ExternalInput")
    bv_d = nc.dram_tensor("bv", [D], BF16, kind="ExternalInput")
    gam_d = nc.dram_tensor("gam", [D], FP32, kind="ExternalInput")
    bet_d = nc.dram_tensor("bet", [D], FP32, kind="ExternalInput")

    y_d = nc.dram_tensor("y", [SL, D], FP32, kind="ExternalOutput")
    # scratch for the softmax-denominator partition broadcast
    zsc_d = nc.dram_tensor("zsc", [PAIRS * 2 * SL], FP32R)

    wq_r = wq_d.rearrange("(c q) m -> q c m", q=P)
    wk_r = wk_d.rearrange("(c q) m -> q c m", q=P)
    wv_r = wv_d.rearrange("(c q) m -> q c m", q=P)
    wo_r = wo_d.rearrange("(c q) m -> q c m", q=P)
    xqt_r = xqt_d.rearrange("(c q) s -> q c s", q=P)
    xkt_r = xkt_d.rearrange("(c q) s -> q c s", q=P)
    xvt_r = xvt_d.rearrange("(c q) s -> q c s", q=P)

    with tile.TileContext(nc) as tc:
        with (
            tc.tile_pool(name="consts", bufs=1) as consts,
            tc.tile_pool(name="big", bufs=1) as big,
            tc.tile_pool(name="xvp", bufs=3) as xvp,
            tc.tile_pool(name="wcol", bufs=3) as wcol,
            tc.tile_pool(name="wres", bufs=1) as wres,
            tc.tile_pool(name="etp", bufs=3) as etp,
            tc.tile_pool(name="normp", bufs=2) as normp,
            tc.tile_pool(name="outp", bufs=2) as outp,
            tc.tile_pool(name="small", bufs=2) as small,
            tc.tile_pool(name="ps", bufs=1, space="PSUM") as ps,
        ):
            # ---- constants + early DMAs for the Q projection ----
            bq_sb = consts.tile([P, KC], FP32)
            nc.sync.dma_start(bq_sb[:], bq_d.rearrange("(m q) -> q m", q=P))
            bk_sb = consts.tile([P, KC], FP32)
            nc.sync.dma_start(bk_sb[:], bk_d.rearrange("(m q) -> q m", q=P))
            xqT = big.tile([P, KC, SL], BF16, tag="xqT")
            nc.sync.dma_start(xqT[:], xqt_r[:])
            # bulk loads on other queues so they stream in parallel
            xkT = big.tile([P, KC, S], BF16, tag="xkT")
            nc.scalar.dma_start(xkT[:], xkt_r[:])
            wv_sb = wres.tile([P, KC, D], BF16, tag="wres")
            nc.scalar.dma_start(wv_sb[:], wv_r[:])

            def bcast_load(src, tag, dt):
                t = consts.tile([P, D], dt, tag=tag)
                ap = bass.AP(tensor=src, offset=0, ap=[[0, P], [1, D]])
                nc.gpsimd.dma_start(out=t[:], in_=ap)
                return t

            bv_b = bcast_load(bv_d, "bv_b", BF16)
            if apply_gb:
                gam_b = bcast_load(gam_d, "gam_b", FP32)
                bet_b = bcast_load(bet_d, "bet_b", FP32)
            eps_t = consts.tile([P, 1], FP32)
            nc.vector.memset(eps_t[:], EPS)

            # ---- PE clock warmup + ACT exp table preload (runs during
            # the input DMA preamble; results are never read) ----
            warm = consts.tile([P, P], BF16)
            nc.vector.memset(warm[:], 0.001)
            wx1 = consts.tile([P, 1], FP32)
            nc.vector.memset(wx1[:], 0.0)
            wxo = consts.tile([P, 1], BF16)
            nc.scalar.activation(out=wxo[:], in_=wx1[:], func=AF.Exp, scale=0.125)
            for _ in range(36):
                pw = ps.tile([P, 2, SL], FP32, tag="mm2", bufs=2)
                nc.tensor.matmul(pw[:, 0, 0:P], warm[:], warm[:], start=True, stop=True)

            # ---- Q^T projection (own rows): qt[q, m, s] = Q^T ----
            qt = big.tile([P, KC, SL], BF16, tag="qt")
            for m in range(KC):
                wq_t = wcol.tile([P, KC, P], BF16, tag="wcol")
                nc.sync.dma_start(wq_t[:], wq_r[:, :, ts(m, P)])
                pq = ps.tile([P, 2, SL], FP32, tag="mm2", bufs=2)
                for k in range(KC):
                    nc.tensor.matmul(
                        pq[:, 0, :],
                        wq_t[:, k, :],
                        xqT[:, k, :],
                        start=(k == 0),
                        stop=(k == KC - 1),
                    )
                nc.scalar.activation(
                    out=qt[:, m, :], in_=pq[:, 0, :], func=AF.Identity,
                    bias=bq_sb[:, m : m + 1],
                )

            # ---- K^T projection (full sequence, SBUF-resident) ----
            ktf = big.tile([P, KC, S], BF16, tag="ktf")
            for m in range(KC):
                wk_t = wcol.tile([P, KC, P], BF16, tag="wcol")
                nc.sync.dma_start(wk_t[:], wk_r[:, :, ts(m, P)])
                for g in range(2):
                    pk = ps.tile([P, 2, SL], FP32, tag="mm2", bufs=2)
                    for k in range(KC):
                        for q2 in range(2):
                            nc.tensor.matmul(
                                pk[:, q2, :],
                                wk_t[:, k, :],
                                xkT[:, k, ds((g * 2 + q2) * SL, SL)],
                                start=(k == 0),
                                stop=(k == KC - 1),
                            )
                    nc.scalar.activation(
                        out=ktf[:, m, ds(g * 1024, 1024)],
                        in_=pk[:].rearrange("q a s -> q (a s)"),
                        func=AF.Identity,
                        bias=bk_sb[:, m : m + 1],
                    )

            # ---- V projection (full sequence, pair-augmented, SBUF) ----
            # vf[q, sc, pair, 130] = [V_even 64 | 1 | V_odd 64 | 1] bf16
            vf = big.tile([P, CH, PAIRS, 130], BF16, tag="vf")
            nc.vector.memset(vf[:, :, :, 64:65], 1.0)
            nc.vector.memset(vf[:, :, :, 129:130], 1.0)
            for sc in range(CH):
                xv_t = xvp.tile([P, KC, P], BF16, tag="xv")
                nc.sync.dma_start(xv_t[:], xvt_r[:, :, ts(sc, P)])
                pv = ps.tile([P, 2, SL], FP32, tag="mm2", bufs=2)
                for k in range(KC):
                    for half in range(2):
                        nc.tensor.matmul(
                            pv[:, half, :],
                            xv_t[:, k, :],
                            wv_sb[:, k, ds(half * 512, 512)],
                            start=(k == 0),
                            stop=(k == KC - 1),
                        )
                for half in range(2):
                    vdst = vf[:, sc, ds(half * 4, 4), :].rearrange(
                        "q pl (j e) -> q pl j e", e=65
                    )
                    nc.vector.tensor_tensor(
                        vdst[:, :, :, 0:64],
                        pv[:, half, :].rearrange("q (pl j e) -> q pl j e", pl=4, j=2),
                        bv_b[:, ds(half * 512, 512)].rearrange(
                            "q (pl j e) -> q pl j e", pl=4, j=2
                        ),
                        ALU.add,
                    )

            # wo into the wres slot (DMA runs during attention, after the
            # last wv read)
            wo_sb = wres.tile([P, KC, D], BF16, tag="wres")
            nc.scalar.dma_start(wo_sb[:], wo_r[:])

            # ---- attention ----
            ctx = big.tile([P, PAIRS, SL], BF16, tag="ctx")

            def emit_normalize(p_, uA, uB):
                # rows 0..63 of ut / row 64 -> ctx[:, p_, :].  The
                # denominator reciprocal is broadcast to 64 partitions by
                # a DRAM bounce (stride-0 partition read) — no PE, no PSUM.
                for j, ut in enumerate((uA, uB)):
                    rec = normp.tile([P, SL], FP32R, tag="rec")
                    with nc.allow_low_precision(
                        reason="float32r is bit-identical to float32 in SBUF"
                    ):
                        nc.vector.reciprocal(out=rec[64:65, :], in_=ut[64:65, :])
                    off = (p_ * 2 + j) * SL
                    nc.sync.dma_start(
                        bass.AP(tensor=zsc_d, offset=off, ap=[[0, 1], [1, SL]]),
                        rec[64:65, :],
                    )
                    bcs = normp.tile([P, SL], FP32R, tag="bcs")
                    nc.sync.dma_start(
                        bcs[0:64, :],
                        bass.AP(tensor=zsc_d, offset=off, ap=[[0, 64], [1, SL]]),
                    )
                    if j == 0:
                        nc.vector.tensor_tensor(
                            ctx[0:64, p_, :], ut[0:64, :], bcs[0:64, :], ALU.mult
                        )
                    else:
                        ctmp = normp.tile([P, SL], BF16, tag="ctmp")
                        nc.vector.tensor_tensor(
                            ctmp[0:64, :], ut[0:64, :], bcs[0:64, :], ALU.mult
                        )
                        # partition shift 0-63 -> 64-127 via SBUF-SBUF DMA
                        nc.gpsimd.dma_start(ctx[64:128, p_, :], ctmp[0:64, :])

            pend = None
            norm_pend = None
            for p in range(PAIRS):
                utA = ps.tile([P, SL], FP32, tag="ut", bufs=4)
                utB = ps.tile([P, SL], FP32, tag="ut", bufs=4)
                for c in range(CH):
                    st2 = ps.tile([P, 2, SL], FP32, tag="mm2", bufs=2)
                    for j in range(2):
                        nc.tensor.matmul(
                            st2[:, j, :],
                            ktf[ds(j * 64, 64), p, ts(c, P)],
                            qt[ds(j * 64, 64), p, :],
                            start=True,
                            stop=True,
                        )
                    et2 = etp.tile([P, 2, SL], BF16, tag="et")
                    nc.scalar.activation(
                        out=et2[:], in_=st2[:], func=AF.Exp, scale=0.125
                    )
                    if pend is not None:
                        pp_, pc, pets, puA, puB = pend
                        for j, ut in enumerate((puA, puB)):
                            nc.tensor.matmul(
                                ut[:65, :],
                                vf[:, pc, pp_, ds(j * 65, 65)],
                                pets[:, j, :],
                                start=(pc == 0),
                                stop=(pc == CH - 1),
                            )
                    if c == 2 and norm_pend is not None:
                        emit_normalize(*norm_pend)
                        norm_pend = None
                    pend = (p, c, et2, utA, utB)
                norm_pend = (p, utA, utB)
            pp_, pc, pets, puA, puB = pend
            for j, ut in enumerate((puA, puB)):
                nc.tensor.matmul(
                    ut[:65, :],
                    vf[:, pc, pp_, ds(j * 65, 65)],
                    pets[:, j, :],
                    start=False,
                    stop=True,
                )
            emit_normalize(*norm_pend)

            # ---- output projection + residual(+bo) + LayerNorm ----
            for i in range(SQ):
                res_t = outp.tile([P, D], FP32, tag="res")
                nc.gpsimd.dma_start(res_t[:], res_d[ts(i, P), :])
                po = ps.tile([P, 2, SL], FP32, tag="mm2", bufs=2)
                for n in range(2):
                    for pp2 in range(PAIRS):
                        nc.tensor.matmul(
                            po[:, n, :],
                            ctx[:, pp2, ts(i, P)],
                            wo_sb[:, pp2, ds(n * 512, 512)],
                            start=(pp2 == 0),
                            stop=(pp2 == PAIRS - 1),
                        )
                orow = outp.tile([P, D], FP32, tag="orow")
                nc.vector.tensor_tensor(
                    orow[:], po[:].rearrange("q a s -> q (a s)"), res_t[:], ALU.add
                )
                stats = small.tile([P, 2, 6], FP32, tag="stats")
                nc.vector.bn_stats(stats[:, 0, :], orow[:, 0:512])
                nc.vector.bn_stats(stats[:, 1, :], orow[:, 512:1024])
                mv = small.tile([P, 2], FP32, tag="mv")
                nc.vector.bn_aggr(mv[:], stats[:])
                std = small.tile([P, 1], FP32, tag="std")
                nc.scalar.activation(
                    out=std[:], in_=mv[:, 1:2], func=AF.Sqrt, bias=eps_t[:], scale=1.0
                )
                rstd = small.tile([P, 1], FP32, tag="rstd")
                nc.vector.reciprocal(out=rstd[:], in_=std[:])
                nmr = small.tile([P, 1], FP32, tag="nmr")
                nc.vector.tensor_scalar(
                    out=nmr[:], in0=mv[:, 0:1], scalar1=rstd[:], scalar2=-1.0,
                    op0=ALU.mult, op1=ALU.mult,
                )
                yt = outp.tile([P, D], FP32, tag="yt")
                nc.scalar.activation(
                    out=yt[:], in_=orow[:], func=AF.Identity,
                    bias=nmr[:], scale=rstd[:],
                )
                if apply_gb:
                    nc.vector.tensor_tensor(yt[:], yt[:], gam_b[:], ALU.mult)
                    nc.vector.tensor_tensor(yt[:], yt[:], bet_b[:], ALU.add)
                nc.sync.dma_start(y_d[ts(i, P), :], yt[:])

    nc.compile()
    return nc


def get_nc(apply_gb: bool):
    key = ("nc", apply_gb)
    if key not in _NC_CACHE:
        _NC_CACHE[key] = build_nc(apply_gb)
    return _NC_CACHE[key]


def kernel(
    query,
    key,
    value,
    Wq,
    bq,
    Wk,
    bk,
    Wv,
    bv,
    Wo,
    bo,
    ln_gamma,
    ln_beta,
    _trace=False,
    _trace_cores=None,
):
    query = np.ascontiguousarray(np.asarray(query, dtype=np.float32))
    key = np.ascontiguousarray(np.asarray(key, dtype=np.float32))
    value = np.ascontiguousarray(np.asarray(value, dtype=np.float32))
    bo_f = np.asarray(bo, np.float32)
    gam_f = np.ascontiguousarray(np.asarray(ln_gamma, np.float32))
    bet_f = np.ascontiguousarray(np.asarray(ln_beta, np.float32))
    apply_gb = not (
        np.all(gam_f == np.float32(1.0)) and np.all(bet_f == np.float32(0.0))
    )
    shared = {
        "wq": np.ascontiguousarray(np.asarray(Wq, np.float32).astype(BFNP)),
        "wk": np.ascontiguousarray(np.asarray(Wk, np.float32).astype(BFNP)),
        "wv": np.ascontiguousarray(np.asarray(Wv, np.float32).astype(BFNP)),
        "wo": np.ascontiguousarray(np.asarray(Wo, np.float32).astype(BFNP)),
        "bq": np.ascontiguousarray(np.asarray(bq, np.float32)),
        "bk": np.ascontiguousarray(np.asarray(bk, np.float32)),
        "bv": np.ascontiguousarray(np.asarray(bv, np.float32).astype(BFNP)),
        "gam": gam_f,
        "bet": bet_f,
    }
    kT = [np.ascontiguousarray(key[b].T.astype(BFNP)) for b in range(B)]
    vT = [np.ascontiguousarray(value[b].T.astype(BFNP)) for b in range(B)]
    in_maps = []
    for c in range(N_CORES):
        b, r = divmod(c, NB)
        rows = slice(r * SL, (r + 1) * SL)
        xq_rows = query[b, rows, :]
        m = dict(shared)
        m["xqt"] = np.ascontiguousarray(xq_rows.T.astype(BFNP))
        m["xkt"] = kT[b]
        m["xvt"] = vT[b]
        m["resg"] = np.ascontiguousarray(xq_rows + bo_f[None, :])
        in_maps.append(m)

    nc = get_nc(apply_gb)
    res = run_bass_kernel_spmd(
        nc,
        in_maps,
        list(range(N_CORES)),
        trace=_trace,
        trace_cores=_trace_cores,
    )
    out = np.empty((B, S, D), dtype=np.float32)
    for c in range(N_CORES):
        b, r = divmod(c, NB)
        out[b, r * SL : (r + 1) * SL, :] = res.results[c]["y"]
    if _trace:
        return out, res
    return out


# revision 18
# speedup vs baseline: 1.6440x; 1.0532x over previous
"""Multi-head attention + residual + LayerNorm on 8 Trainium2 NeuronCores.

Sharding: core c in 0..7 handles batch b = c//4 and query-row quarter
r = c%4 (rows 512r..512r+512 of S=2048), with ALL 16 heads.  key/value
are replicated per batch (host-side staging); each core computes the
full-sequence K^T and V projections itself — measured collectives on
this stack cost ~130us per 2MB AllGather, far more than the redundant
PE work, and the local pipeline keeps the PE clock warm.

v3 (vs 542us fp32r baseline, 408us v2):
  - host stages x^T (pre-transposed) and all matmul operands in bf16:
    no PE transposes, FWL weight loads, half the DMA bytes
  - K^T, V_aug, Q^T, ctx all SBUF-resident (no DRAM roundtrips)
  - all PSUM matmul tiles are [128, 1024] 2-bank tiles (tag mm2 ring 2
    + softmax accumulators ut ring 4 = 8 banks): projections pair two
    512-col accumulators per tile and evacuate with ONE wide ACT op;
    attention computes both heads' scores into one tile and exps them
    with ONE 1024-wide ACTIVATE (the ACT 352-cycle/instr overhead was
    pacing the attention phase at 1440ns/chunk vs PE's ~1000ns)
  - softmax denominator broadcast via DRAM-bounce DMA (partition-
    stride-0 read) instead of a PE matmul: normalize is entirely off
    the PE critical path, so pairs pipeline without stalls
  - attention software pipeline crosses pair boundaries (U^T matmuls
    of chunk c issue during chunk c+1, last chunk drains into the next
    pair's first chunk)
  - bulk DMAs ride separate engine queues so the Q-projection feed is
    not queued behind the 4MB K/V loads
  - ~36 warmup matmuls + a dummy exp during the DMA preamble warm the
    PE clock (HAM) and preload the ACT exp table
  - LayerNorm: residual+bo folded on host, normalization via one ACT
    op with per-partition scale/bias; gamma/beta applied only when
    they are non-trivial (checked on host, separate compiled variant)

Accumulations stay fp32 in PSUM; softmax reciprocal / LN stats fp32.
"""

import sys

if "/opt/trn_rl_repo" not in sys.path:
    sys.path.insert(0, "/opt/trn_rl_repo")

import ml_dtypes
import numpy as np

import concourse.bacc as bacc
import concourse.bass as bass
import concourse.mybir as mybir
import concourse.tile as tile
from concourse.bass import ds, ts
from concourse.bass_utils import run_bass_kernel_spmd

FP32R = mybir.dt.float32r
FP32 = mybir.dt.float32
BF16 = mybir.dt.bfloat16
FP8 = mybir.dt.float8e4
AF = mybir.ActivationFunctionType
ALU = mybir.AluOpType
DR = mybir.MatmulPerfMode.DoubleRow
BFNP = ml_dtypes.bfloat16
FP8NP = ml_dtypes.float8_e4m3
# exp(s/8 - EXP_SHIFT) keeps softmax weights inside fp8e4 range (max 240);
# the constant shift cancels exactly in the normalize ratio.
EXP_SHIFT = -3.0

N_CORES = 8
B = 2
S = 2048
D = 1024
H = 16
DK = 64
P = 128

SL = S // 4  # 512 local query rows per core
KC = D // P  # 8 contraction chunks over d_model
SQ = SL // P  # 4 sq subchunks of 128 (per 512-row block)
CH = S // P  # 16 sk chunks
PAIRS = H // 2  # 8 head pairs
NB = 4  # row quarters
EPS = 1e-5

_NC_CACHE = {}


def build_nc(apply_gb: bool):
    nc = bacc.Bacc(num_devices=N_CORES)

    xqt_d = nc.dram_tensor("xqt", [D, SL], FP8, kind="ExternalInput")
    xkt_d = nc.dram_tensor("xkt", [D, S], FP8, kind="ExternalInput")
    xvt_d = nc.dram_tensor("xvt", [D, S], FP8, kind="ExternalInput")
    res_d = nc.dram_tensor("resg", [SL, D], FP32, kind="ExternalInput")
    wq_d = nc.dram_tensor("wq", [D, D], FP8, kind="ExternalInput")
    wk_d = nc.dram_tensor("wk", [D, D], FP8, kind="ExternalInput")
    wv_d = nc.dram_tensor("wv", [D, D], FP8, kind="ExternalInput")
    wo_d = nc.dram_tensor("wo", [D, D], BF16, kind="ExternalInput")
    bq_d = nc.dram_tensor("bq", [D], FP32, kind="ExternalInput")
    bk_d = nc.dram_tensor("bk", [D], FP32, kind="ExternalInput")
    bv_d = nc.dram_tensor("bv", [D], FP32, kind="ExternalInput")
    gam_d = nc.dram_tensor("gam", [D], FP32, kind="ExternalInput")
    bet_d = nc.dram_tensor("bet", [D], FP32, kind="ExternalInput")

    y_d = nc.dram_tensor("y", [SL, D], FP32, kind="ExternalOutput")
    # scratch for the softmax-denominator partition broadcast
    zsc_d = nc.dram_tensor("zsc", [PAIRS * 2 * SL], FP32R)

    wq_r = wq_d.rearrange("(c q) m -> q c m", q=P)
    wk_r = wk_d.rearrange("(c q) m -> q c m", q=P)
    wv_r = wv_d.rearrange("(c q) m -> q c m", q=P)
    wo_r = wo_d.rearrange("(c q) m -> q c m", q=P)
    xqt_r = xqt_d.rearrange("(c q) s -> q c s", q=P)
    xkt_r = xkt_d.rearrange("(c q) s -> q c s", q=P)
    xvt_r = xvt_d.rearrange("(c q) s -> q c s", q=P)

    with tile.TileContext(nc) as tc:
        with (
            tc.tile_pool(name="consts", bufs=1) as consts,
            tc.tile_pool(name="big", bufs=1) as big,
            tc.tile_pool(name="xvp", bufs=3) as xvp,
            tc.tile_pool(name="wcol", bufs=3) as wcol,
            tc.tile_pool(name="wres", bufs=1) as wres,
            tc.tile_pool(name="etp", bufs=3) as etp,
            tc.tile_pool(name="normp", bufs=2) as normp,
            tc.tile_pool(name="outp", bufs=2) as outp,
            tc.tile_pool(name="small", bufs=2) as small,
            tc.tile_pool(name="ps", bufs=1, space="PSUM") as ps,
        ):
            # ---- constants + early DMAs for the Q projection ----
            bq_sb = consts.tile([P, KC], FP32)
            nc.sync.dma_start(bq_sb[:], bq_d.rearrange("(m q) -> q m", q=P))
            bk_sb = consts.tile([P, KC], FP32)
            nc.sync.dma_start(bk_sb[:], bk_d.rearrange("(m q) -> q m", q=P))
            xqT = big.tile([P, KC, SL], FP8, tag="xqT")
            nc.sync.dma_start(xqT[:], xqt_r[:])
            # bulk loads on other queues so they stream in parallel
            xkT = big.tile([P, KC, S], FP8, tag="xkT")
            nc.scalar.dma_start(xkT[:], xkt_r[:])
            wv_sb = wres.tile([P, KC, D], FP8, tag="wres8")
            nc.scalar.dma_start(wv_sb[:], wv_r[:])

            def bcast_load(src, tag, dt):
                t = consts.tile([P, D], dt, tag=tag)
                ap = bass.AP(tensor=src, offset=0, ap=[[0, P], [1, D]])
                nc.gpsimd.dma_start(out=t[:], in_=ap)
                return t

            bv_b = bcast_load(bv_d, "bv_b", FP32)
            if apply_gb:
                gam_b = bcast_load(gam_d, "gam_b", FP32)
                bet_b = bcast_load(bet_d, "bet_b", FP32)
            eps_t = consts.tile([P, 1], FP32)
            nc.vector.memset(eps_t[:], EPS)
            shf_t = consts.tile([P, 1], FP32)
            nc.vector.memset(shf_t[:], EXP_SHIFT)

            # ---- PE clock warmup + ACT exp table preload (runs during
            # the input DMA preamble; results are never read) ----
            warm = consts.tile([P, P], BF16)
            nc.vector.memset(warm[:], 0.001)
            wx1 = consts.tile([P, 1], FP32)
            nc.vector.memset(wx1[:], 0.0)
            wxo = consts.tile([P, 1], BF16)
            nc.scalar.activation(out=wxo[:], in_=wx1[:], func=AF.Exp, scale=0.125)
            for _ in range(36):
                pw = ps.tile([P, 2, SL], FP32, tag="mm2", bufs=2)
                nc.tensor.matmul(pw[:, 0, 0:P], warm[:], warm[:], start=True, stop=True)

            # ---- Q^T projection (own rows): qt[q, m, s] = Q^T ----
            qt = big.tile([P, KC, SL], BF16, tag="qt")
            for m in range(KC):
                wq_t = wcol.tile([P, KC, P], FP8, tag="wcol")
                nc.sync.dma_start(wq_t[:], wq_r[:, :, ts(m, P)])
                pq = ps.tile([P, 2, SL], FP32, tag="mm2", bufs=2)
                for k in range(0, KC, 2):
                    nc.tensor.matmul(
                        pq[:, 0, :],
                        wq_t[:, k : k + 2, :],
                        xqT[:, k : k + 2, :],
                        start=(k == 0),
                        stop=(k == KC - 2),
                        perf_mode=DR,
                    )
                nc.scalar.activation(
                    out=qt[:, m, :], in_=pq[:, 0, :], func=AF.Identity,
                    bias=bq_sb[:, m : m + 1],
                )

            # ---- K^T projection (full sequence, SBUF-resident) ----
            ktf = big.tile([P, KC, S], BF16, tag="ktf")
            for m in range(KC):
                wk_t = wcol.tile([P, KC, P], FP8, tag="wcol")
                nc.sync.dma_start(wk_t[:], wk_r[:, :, ts(m, P)])
                for g in range(2):
                    pk = ps.tile([P, 2, SL], FP32, tag="mm2", bufs=2)
                    for k in range(0, KC, 2):
                        for q2 in range(2):
                            nc.tensor.matmul(
                                pk[:, q2, :],
                                wk_t[:, k : k + 2, :],
                                xkT[:, k : k + 2, ds((g * 2 + q2) * SL, SL)],
                                start=(k == 0),
                                stop=(k == KC - 2),
                                perf_mode=DR,
                            )
                    nc.scalar.activation(
                        out=ktf[:, m, ds(g * 1024, 1024)],
                        in_=pk[:].rearrange("q a s -> q (a s)"),
                        func=AF.Identity,
                        bias=bk_sb[:, m : m + 1],
                    )

            # ---- V projection (full sequence, pair-augmented, SBUF) ----
            # vf[q, sc, pair, 130] = [V_even 64 | 1 | V_odd 64 | 1] bf16
            vf = big.tile([P, CH, PAIRS, 130], FP8, tag="vf")
            nc.vector.memset(vf[:, :, :, 64:65], 1.0)
            nc.vector.memset(vf[:, :, :, 129:130], 1.0)
            for sc in range(CH):
                xv_t = xvp.tile([P, KC, P], FP8, tag="xv")
                nc.sync.dma_start(xv_t[:], xvt_r[:, :, ts(sc, P)])
                pv = ps.tile([P, 2, SL], FP32, tag="mm2", bufs=2)
                for k in range(0, KC, 2):
                    for half in range(2):
                        nc.tensor.matmul(
                            pv[:, half, :],
                            xv_t[:, k : k + 2, :],
                            wv_sb[:, k : k + 2, ds(half * 512, 512)],
                            start=(k == 0),
                            stop=(k == KC - 2),
                            perf_mode=DR,
                        )
                for half in range(2):
                    vdst = vf[:, sc, ds(half * 4, 4), :].rearrange(
                        "q pl (j e) -> q pl j e", e=65
                    )
                    nc.vector.tensor_tensor(
                        vdst[:, :, :, 0:64],
                        pv[:, half, :].rearrange("q (pl j e) -> q pl j e", pl=4, j=2),
                        bv_b[:, ds(half * 512, 512)].rearrange(
                            "q (pl j e) -> q pl j e", pl=4, j=2
                        ),
                        ALU.add,
                    )

            # wo into the wres slot (DMA runs during attention, after the
            # last wv read)
            wo_sb = wres.tile([P, KC, D], BF16, tag="wres")
            nc.scalar.dma_start(wo_sb[:], wo_r[:])

            # ---- attention ----
            ctx = big.tile([P, PAIRS, SL], BF16, tag="ctx")

            def emit_normalize(p_, uA, uB):
                # rows 0..63 of ut / row 64 -> ctx[:, p_, :].  The
                # denominator reciprocal is broadcast to 64 partitions by
                # a DRAM bounce (stride-0 partition read) — no PE, no PSUM.
                for j, ut in enumerate((uA, uB)):
                    rec = normp.tile([P, SL], FP32R, tag="rec")
                    with nc.allow_low_precision(
                        reason="float32r is bit-identical to float32 in SBUF"
                    ):
                        nc.vector.reciprocal(out=rec[64:65, :], in_=ut[64:65, :])
                    off = (p_ * 2 + j) * SL
                    nc.sync.dma_start(
                        bass.AP(tensor=zsc_d, offset=off, ap=[[0, 1], [1, SL]]),
                        rec[64:65, :],
                    )
                    bcs = normp.tile([P, SL], FP32R, tag="bcs")
                    nc.sync.dma_start(
                        bcs[0:64, :],
                        bass.AP(tensor=zsc_d, offset=off, ap=[[0, 64], [1, SL]]),
                    )
                    if j == 0:
                        nc.vector.tensor_tensor(
                            ctx[0:64, p_, :], ut[0:64, :], bcs[0:64, :], ALU.mult
                        )
                    else:
                        ctmp = normp.tile([P, SL], BF16, tag="ctmp")
                        nc.vector.tensor_tensor(
                            ctmp[0:64, :], ut[0:64, :], bcs[0:64, :], ALU.mult
                        )
                        # partition shift 0-63 -> 64-127 via SBUF-SBUF DMA
                        nc.gpsimd.dma_start(ctx[64:128, p_, :], ctmp[0:64, :])

            def flush_u(pp_, pc, pet4, puA, puB):
                # U^T accumulation over the key-chunk pair (pc-1, pc) with
                # one fp8 DoubleRow matmul per head
                for j, ut in enumerate((puA, puB)):
                    nc.tensor.matmul(
                        ut[:65, :],
                        vf[:, pc - 1 : pc + 1, pp_, ds(j * 65, 65)],
                        pet4[:, :, j, :],
                        start=(pc == 1),
                        stop=(pc == CH - 1),
                        perf_mode=DR,
                    )

            pend = None
            norm_pend = None
            et4 = None
            for p in range(PAIRS):
                utA = ps.tile([P, SL], FP32, tag="ut", bufs=4)
                utB = ps.tile([P, SL], FP32, tag="ut", bufs=4)
                for c in range(CH):
                    if c % 2 == 0:
                        et4 = etp.tile([P, 2, 2, SL], FP8, tag="et")
                    st2 = ps.tile([P, 2, SL], FP32, tag="mm2", bufs=2)
                    for j in range(2):
                        nc.tensor.matmul(
                            st2[:, j, :],
                            ktf[ds(j * 64, 64), p, ts(c, P)],
                            qt[ds(j * 64, 64), p, :],
                            start=True,
                            stop=True,
                        )
                    nc.scalar.activation(
                        out=et4[:, c % 2, :, :], in_=st2[:], func=AF.Exp,
                        scale=0.125, bias=shf_t[:],
                    )
                    if pend is not None:
                        flush_u(*pend)
                        pend = None
                    if c == 2 and norm_pend is not None:
                        emit_normalize(*norm_pend)
                        norm_pend = None
                    if c % 2 == 1:
                        pend = (p, c, et4, utA, utB)
                norm_pend = (p, utA, utB)
            flush_u(*pend)
            pend = None
            emit_normalize(*norm_pend)

            # ---- output projection + residual(+bo) + LayerNorm ----
            for i in range(SQ):
                res_t = outp.tile([P, D], FP32, tag="res")
                nc.gpsimd.dma_start(res_t[:], res_d[ts(i, P), :])
                po = ps.tile([P, 2, SL], FP32, tag="mm2", bufs=2)
                for n in range(2):
                    for pp2 in range(PAIRS):
                        nc.tensor.matmul(
                            po[:, n, :],
                            ctx[:, pp2, ts(i, P)],
                            wo_sb[:, pp2, ds(n * 512, 512)],
                            start=(pp2 == 0),
                            stop=(pp2 == PAIRS - 1),
                        )
                orow = outp.tile([P, D], FP32, tag="orow")
                nc.vector.tensor_tensor(
                    orow[:], po[:].rearrange("q a s -> q (a s)"), res_t[:], ALU.add
                )
                stats = small.tile([P, 2, 6], FP32, tag="stats")
                nc.vector.bn_stats(stats[:, 0, :], orow[:, 0:512])
                nc.vector.bn_stats(stats[:, 1, :], orow[:, 512:1024])
                mv = small.tile([P, 2], FP32, tag="mv")
                nc.vector.bn_aggr(mv[:], stats[:])
                std = small.tile([P, 1], FP32, tag="std")
                nc.scalar.activation(
                    out=std[:], in_=mv[:, 1:2], func=AF.Sqrt, bias=eps_t[:], scale=1.0
                )
                rstd = small.tile([P, 1], FP32, tag="rstd")
                nc.vector.reciprocal(out=rstd[:], in_=std[:])
                nmr = small.tile([P, 1], FP32, tag="nmr")
                nc.vector.tensor_scalar(
                    out=nmr[:], in0=mv[:, 0:1], scalar1=rstd[:], scalar2=-1.0,
                    op0=ALU.mult, op1=ALU.mult,
                )
                yt = outp.tile([P, D], FP32, tag="yt")
                nc.scalar.activation(
                    out=yt[:], in_=orow[:], func=AF.Identity,
                    bias=nmr[:], scale=rstd[:],
                )
                if apply_gb:
                    nc.vector.tensor_tensor(yt[:], yt[:], gam_b[:], ALU.mult)
                    nc.vector.tensor_tensor(yt[:], yt[:], bet_b[:], ALU.add)
                nc.sync.dma_start(y_d[ts(i, P), :], yt[:])

    nc.compile()
    return nc


def get_nc(apply_gb: bool):
    key = ("nc", apply_gb)
    if key not in _NC_CACHE:
        _NC_CACHE[key] = build_nc(apply_gb)
    return _NC_CACHE[key]


def kernel(
    query,
    key,
    value,
    Wq,
    bq,
    Wk,
    bk,
    Wv,
    bv,
    Wo,
    bo,
    ln_gamma,
    ln_beta,
    _trace=False,
    _trace_cores=None,
):
    query = np.ascontiguousarray(np.asarray(query, dtype=np.float32))
    key = np.ascontiguousarray(np.asarray(key, dtype=np.float32))
    value = np.ascontiguousarray(np.asarray(value, dtype=np.float32))
    bo_f = np.asarray(bo, np.float32)
    gam_f = np.ascontiguousarray(np.asarray(ln_gamma, np.float32))
    bet_f = np.ascontiguousarray(np.asarray(ln_beta, np.float32))
    apply_gb = not (
        np.all(gam_f == np.float32(1.0)) and np.all(bet_f == np.float32(0.0))
    )
    shared = {
        "wq": np.ascontiguousarray(np.asarray(Wq, np.float32).astype(FP8NP)),
        "wk": np.ascontiguousarray(np.asarray(Wk, np.float32).astype(FP8NP)),
        "wv": np.ascontiguousarray(np.asarray(Wv, np.float32).astype(FP8NP)),
        "wo": np.ascontiguousarray(np.asarray(Wo, np.float32).astype(BFNP)),
        "bq": np.ascontiguousarray(np.asarray(bq, np.float32)),
        "bk": np.ascontiguousarray(np.asarray(bk, np.float32)),
        "bv": np.ascontiguousarray(np.asarray(bv, np.float32)),
        "gam": gam_f,
        "bet": bet_f,
    }
    kT = [np.ascontiguousarray(key[b].T.astype(FP8NP)) for b in range(B)]
    vT = [np.ascontiguousarray(value[b].T.astype(FP8NP)) for b in range(B)]
    in_maps = []
    for c in range(N_CORES):
        b, r = divmod(c, NB)
        rows = slice(r * SL, (r + 1) * SL)
        xq_rows = query[b, rows, :]
        m = dict(shared)
        m["xqt"] = np.ascontiguousarray(xq_rows.T.astype(FP8NP))
        m["xkt"] = kT[b]
        m["xvt"] = vT[b]
        m["resg"] = np.ascontiguousarray(xq_rows + bo_f[None, :])
        in_maps.append(m)

    nc = get_nc(apply_gb)
    res = run_bass_kernel_spmd(
        nc,
        in_maps,
        list(range(N_CORES)),
        trace=_trace,
        trace_cores=_trace_cores,
    )
    out = np.empty((B, S, D), dtype=np.float32)
    for c in range(N_CORES):
        b, r = divmod(c, NB)
        out[b, r * SL : (r + 1) * SL, :] = res.results[c]["y"]
    if _trace:
        return out, res
    return out


# revision 28
# speedup vs baseline: 1.8788x; 1.1428x over previous
"""Multi-head attention + residual + LayerNorm on 8 Trainium2 NeuronCores.

Sharding: core c in 0..7 handles batch b = c//4 and query-row quarter
r = c%4 (rows 512r..512r+512 of S=2048), with ALL 16 heads.  key/value
are replicated per batch (host-side staging); each core computes the
full-sequence K^T and V projections itself — measured collectives on
this stack cost ~130us per 2MB AllGather, far more than the redundant
PE work, and the local pipeline keeps the PE clock warm.

v3 (vs 542us fp32r baseline, 408us v2):
  - host stages x^T (pre-transposed) and all matmul operands in bf16:
    no PE transposes, FWL weight loads, half the DMA bytes
  - K^T, V_aug, Q^T, ctx all SBUF-resident (no DRAM roundtrips)
  - all PSUM matmul tiles are [128, 1024] 2-bank tiles (tag mm2 ring 2
    + softmax accumulators ut ring 4 = 8 banks): projections pair two
    512-col accumulators per tile and evacuate with ONE wide ACT op;
    attention computes both heads' scores into one tile and exps them
    with ONE 1024-wide ACTIVATE (the ACT 352-cycle/instr overhead was
    pacing the attention phase at 1440ns/chunk vs PE's ~1000ns)
  - softmax denominator broadcast via DRAM-bounce DMA (partition-
    stride-0 read) instead of a PE matmul: normalize is entirely off
    the PE critical path, so pairs pipeline without stalls
  - attention software pipeline crosses pair boundaries (U^T matmuls
    of chunk c issue during chunk c+1, last chunk drains into the next
    pair's first chunk)
  - bulk DMAs ride separate engine queues so the Q-projection feed is
    not queued behind the 4MB K/V loads
  - ~36 warmup matmuls + a dummy exp during the DMA preamble warm the
    PE clock (HAM) and preload the ACT exp table
  - LayerNorm: residual+bo folded on host, normalization via one ACT
    op with per-partition scale/bias; gamma/beta applied only when
    they are non-trivial (checked on host, separate compiled variant)

Accumulations stay fp32 in PSUM; softmax reciprocal / LN stats fp32.
"""

import sys

if "/opt/trn_rl_repo" not in sys.path:
    sys.path.insert(0, "/opt/trn_rl_repo")

import ml_dtypes
import numpy as np

import concourse.bacc as bacc
import concourse.bass as bass
import concourse.mybir as mybir
import concourse.tile as tile
from concourse.bass import ds, ts
from concourse.bass_utils import run_bass_kernel_spmd

FP32R = mybir.dt.float32r
FP32 = mybir.dt.float32
BF16 = mybir.dt.bfloat16
FP8 = mybir.dt.float8e4
AF = mybir.ActivationFunctionType
ALU = mybir.AluOpType
DR = mybir.MatmulPerfMode.DoubleRow
BFNP = ml_dtypes.bfloat16
FP8NP = ml_dtypes.float8_e4m3
# exp(s/8 - EXP_SHIFT) keeps softmax weights inside fp8e4 range (max 240);
# the constant shift cancels exactly in the normalize ratio.
EXP_SHIFT = -3.0

N_CORES = 8
B = 2
S = 2048
D = 1024
H = 16
DK = 64
P = 128

SL = S // 4  # 512 local query rows per core
KC = D // P  # 8 contraction chunks over d_model
SQ = SL // P  # 4 sq subchunks of 128 (per 512-row block)
CH = S // P  # 16 sk chunks
PAIRS = H // 2  # 8 head pairs
NB = 4  # row quarters
EPS = 1e-5

_NC_CACHE = {}


def build_nc(apply_gb: bool):
    nc = bacc.Bacc(num_devices=N_CORES)

    xqt_d = nc.dram_tensor("xqt", [D, SL], FP8, kind="ExternalInput")
    xkt_d = nc.dram_tensor("xkt", [D, S], FP8, kind="ExternalInput")
    xvt_d = nc.dram_tensor("xvt", [D, S], FP8, kind="ExternalInput")
    res_d = nc.dram_tensor("resg", [SL, D], FP32, kind="ExternalInput")
    wq_d = nc.dram_tensor("wq", [D, D], FP8, kind="ExternalInput")
    wk_d = nc.dram_tensor("wk", [D, D], FP8, kind="ExternalInput")
    wv_d = nc.dram_tensor("wv", [D, D], FP8, kind="ExternalInput")
    wo_d = nc.dram_tensor("wo", [D, D], BF16, kind="ExternalInput")
    bq_d = nc.dram_tensor("bq", [D], FP32, kind="ExternalInput")
    bk_d = nc.dram_tensor("bk", [D], FP32, kind="ExternalInput")
    bv_d = nc.dram_tensor("bv", [D], FP32, kind="ExternalInput")
    gam_d = nc.dram_tensor("gam", [D], FP32, kind="ExternalInput")
    bet_d = nc.dram_tensor("bet", [D], FP32, kind="ExternalInput")

    ones_d = nc.dram_tensor("ones", [P, 64], FP32R, kind="ExternalInput")
    y_d = nc.dram_tensor("y", [SL, D], FP32, kind="ExternalOutput")
    # scratch for the softmax-denominator partition broadcast
    zsc_d = nc.dram_tensor("zsc", [PAIRS * 2 * SL], FP32R)

    wq_r = wq_d.rearrange("(c q) m -> q c m", q=P)
    wk_r = wk_d.rearrange("(c q) m -> q c m", q=P)
    wv_r = wv_d.rearrange("(c q) m -> q c m", q=P)
    wo_r = wo_d.rearrange("(c q) m -> q c m", q=P)
    xqt_r = xqt_d.rearrange("(c q) s -> q c s", q=P)
    xkt_r = xkt_d.rearrange("(c q) s -> q c s", q=P)
    xvt_r = xvt_d.rearrange("(c q) s -> q c s", q=P)

    with tile.TileContext(nc) as tc:
        with (
            tc.tile_pool(name="consts", bufs=1) as consts,
            tc.tile_pool(name="big", bufs=1) as big,
            tc.tile_pool(name="xvp", bufs=3) as xvp,
            tc.tile_pool(name="wcol", bufs=3) as wcol,
            tc.tile_pool(name="wres", bufs=1) as wres,
            tc.tile_pool(name="etp", bufs=3) as etp,
            tc.tile_pool(name="normp", bufs=2) as normp,
            tc.tile_pool(name="outp", bufs=2) as outp,
            tc.tile_pool(name="small", bufs=2) as small,
            tc.tile_pool(name="ps", bufs=1, space="PSUM") as ps,
        ):
            # ---- constants + early DMAs for the Q projection ----
            bq_sb = consts.tile([P, KC], FP32)
            nc.sync.dma_start(bq_sb[:], bq_d.rearrange("(m q) -> q m", q=P))
            bk_sb = consts.tile([P, KC], FP32)
            nc.sync.dma_start(bk_sb[:], bk_d.rearrange("(m q) -> q m", q=P))
            xqT = big.tile([P, KC, SL], FP8, tag="xqT")
            nc.sync.dma_start(xqT[:, 0:4, :], xqt_r[:, 0:4, :])
            nc.gpsimd.dma_start(xqT[:, 4:8, :], xqt_r[:, 4:8, :])
            # bulk loads on other queues so they stream in parallel
            xkT = big.tile([P, KC, S], FP8, tag="xkT")
            nc.scalar.dma_start(xkT[:], xkt_r[:])
            wv_sb = wres.tile([P, KC, D], FP8, tag="wres8")
            nc.scalar.dma_start(wv_sb[:], wv_r[:])

            def bcast_load(src, tag, dt):
                t = consts.tile([P, D], dt, tag=tag)
                ap = bass.AP(tensor=src, offset=0, ap=[[0, P], [1, D]])
                nc.gpsimd.dma_start(out=t[:], in_=ap)
                return t

            bv_b = bcast_load(bv_d, "bv_b", FP32)
            if apply_gb:
                gam_b = bcast_load(gam_d, "gam_b", FP32)
                bet_b = bcast_load(bet_d, "bet_b", FP32)
            eps_t = consts.tile([P, 1], FP32)
            nc.vector.memset(eps_t[:], EPS)
            on64 = consts.tile([P, 64], FP32R)
            nc.sync.dma_start(on64[:], ones_d[:])

            # ---- PE clock warmup + ACT exp table preload (runs during
            # the input DMA preamble; results are never read) ----
            warm = consts.tile([P, P], BF16)
            nc.vector.memset(warm[:], 0.001)
            wx1 = consts.tile([P, 1], FP32)
            nc.vector.memset(wx1[:], 0.0)
            wxo = consts.tile([P, 1], BF16)
            nc.scalar.activation(out=wxo[:], in_=wx1[:], func=AF.Exp, scale=0.125)
            for _ in range(52):
                pw = ps.tile([P, 2, SL], FP32, tag="mm2", bufs=2)
                nc.tensor.matmul(pw[:, 0, 0:P], warm[:], warm[:], start=True, stop=True)

            # ---- Q^T projection (own rows): qt[q, m, s] = Q^T ----
            qt = big.tile([P, KC, SL], BF16, tag="qt")
            for m in range(KC):
                wq_t = wcol.tile([P, KC, P], FP8, tag="wcol")
                nc.sync.dma_start(wq_t[:], wq_r[:, :, ts(m, P)])
                pq = ps.tile([P, 2, SL], FP32, tag="mm2", bufs=2)
                for k in range(0, KC, 2):
                    nc.tensor.matmul(
                        pq[:, 0, :],
                        wq_t[:, k : k + 2, :],
                        xqT[:, k : k + 2, :],
                        start=(k == 0),
                        stop=(k == KC - 2),
                        perf_mode=DR,
                    )
                nc.scalar.activation(
                    out=qt[:, m, :], in_=pq[:, 0, :], func=AF.Identity,
                    bias=bq_sb[:, m : m + 1],
                )

            # ---- K^T projection (full sequence, SBUF-resident) ----
            ktf = big.tile([P, KC, S], BF16, tag="ktf")
            for m in range(KC):
                wk_t = wcol.tile([P, KC, P], FP8, tag="wcol")
                nc.sync.dma_start(wk_t[:], wk_r[:, :, ts(m, P)])
                for g in range(2):
                    pk = ps.tile([P, 2, SL], FP32, tag="mm2", bufs=2)
                    for k in range(0, KC, 2):
                        for q2 in range(2):
                            nc.tensor.matmul(
                                pk[:, q2, :],
                                wk_t[:, k : k + 2, :],
                                xkT[:, k : k + 2, ds((g * 2 + q2) * SL, SL)],
                                start=(k == 0),
                                stop=(k == KC - 2),
                                perf_mode=DR,
                            )
                    nc.scalar.activation(
                        out=ktf[:, m, ds(g * 1024, 1024)],
                        in_=pk[:].rearrange("q a s -> q (a s)"),
                        func=AF.Identity,
                        bias=bk_sb[:, m : m + 1],
                    )

            # ---- V projection (full sequence, pair-augmented, SBUF) ----
            # vf[q, sc, pair, 130] = [V_even 64 | 1 | V_odd 64 | 1] bf16
            vf = big.tile([P, CH, PAIRS, 130], BF16, tag="vf")
            nc.vector.memset(vf[:, :, :, 64:65], 1.0)
            nc.vector.memset(vf[:, :, :, 129:130], 1.0)
            for sc in range(CH):
                xv_t = xvp.tile([P, KC, P], FP8, tag="xv")
                nc.sync.dma_start(xv_t[:], xvt_r[:, :, ts(sc, P)])
                pv = ps.tile([P, 2, SL], FP32, tag="mm2", bufs=2)
                for k in range(0, KC, 2):
                    for half in range(2):
                        nc.tensor.matmul(
                            pv[:, half, :],
                            xv_t[:, k : k + 2, :],
                            wv_sb[:, k : k + 2, ds(half * 512, 512)],
                            start=(k == 0),
                            stop=(k == KC - 2),
                            perf_mode=DR,
                        )
                for half in range(2):
                    vdst = vf[:, sc, ds(half * 4, 4), :].rearrange(
                        "q pl (j e) -> q pl j e", e=65
                    )
                    nc.vector.tensor_tensor(
                        vdst[:, :, :, 0:64],
                        pv[:, half, :].rearrange("q (pl j e) -> q pl j e", pl=4, j=2),
                        bv_b[:, ds(half * 512, 512)].rearrange(
                            "q (pl j e) -> q pl j e", pl=4, j=2
                        ),
                        ALU.add,
                    )

            # wo into the wres slot (DMA runs during attention, after the
            # last wv read)
            wo_sb = wres.tile([P, KC, D], BF16, tag="wres")
            nc.scalar.dma_start(wo_sb[:], wo_r[:])

            # ---- attention ----
            ctx = big.tile([P, PAIRS, SL], BF16, tag="ctx")

            def emit_normalize(p_, uA, uB, fast=False):
                # rows 0..63 of ut / row 64 -> ctx[:, p_, :].  The
                # denominator reciprocal is broadcast to 64 partitions by
                # a DRAM bounce (stride-0 partition read) — no PE, no PSUM.
                # The last pair uses a PE broadcast instead: the bounce
                # latency (~12us) would gate the output projection.
                for j, ut in enumerate((uA, uB)):
                    rec = normp.tile([P, SL], FP32R, tag="rec")
                    with nc.allow_low_precision(
                        reason="float32r is bit-identical to float32 in SBUF"
                    ):
                        nc.vector.reciprocal(out=rec[64:65, :], in_=ut[64:65, :])
                    bcs = normp.tile([P, SL], FP32R, tag="bcs")
                    if fast:
                        bcp = ps.tile([P, 2, SL], FP32, tag="mm2", bufs=2)
                        nc.tensor.matmul(
                            bcp[0:64, 0, :],
                            on64[64:65, :],
                            rec[64:65, :],
                            start=True,
                            stop=True,
                        )
                        nc.vector.tensor_copy(bcs[0:64, :], bcp[0:64, 0, :])
                    else:
                        off = (p_ * 2 + j) * SL
                        nc.sync.dma_start(
                            bass.AP(tensor=zsc_d, offset=off, ap=[[0, 1], [1, SL]]),
                            rec[64:65, :],
                        )
                        nc.sync.dma_start(
                            bcs[0:64, :],
                            bass.AP(tensor=zsc_d, offset=off, ap=[[0, 64], [1, SL]]),
                        )
                    if j == 0:
                        nc.vector.tensor_tensor(
                            ctx[0:64, p_, :], ut[0:64, :], bcs[0:64, :], ALU.mult
                        )
                    else:
                        ctmp = normp.tile([P, SL], BF16, tag="ctmp")
                        nc.vector.tensor_tensor(
                            ctmp[0:64, :], ut[0:64, :], bcs[0:64, :], ALU.mult
                        )
                        # partition shift 0-63 -> 64-127 via SBUF-SBUF DMA
                        nc.gpsimd.dma_start(ctx[64:128, p_, :], ctmp[0:64, :])

            pend = None
            norm_pend = None
            for p in range(PAIRS):
                utA = ps.tile([P, SL], FP32, tag="ut", bufs=4)
                utB = ps.tile([P, SL], FP32, tag="ut", bufs=4)
                for c in range(CH):
                    st2 = ps.tile([P, 2, SL], FP32, tag="mm2", bufs=2)
                    for j in range(2):
                        nc.tensor.matmul(
                            st2[:, j, :],
                            ktf[ds(j * 64, 64), p, ts(c, P)],
                            qt[ds(j * 64, 64), p, :],
                            start=True,
                            stop=True,
                        )
                    et2 = etp.tile([P, 2, SL], BF16, tag="et")
                    nc.scalar.activation(
                        out=et2[:], in_=st2[:], func=AF.Exp, scale=0.125
                    )
                    if pend is not None:
                        pp_, pc, pets, puA, puB = pend
                        for j, ut in enumerate((puA, puB)):
                            nc.tensor.matmul(
                                ut[:65, :],
                                vf[:, pc, pp_, ds(j * 65, 65)],
                                pets[:, j, :],
                                start=(pc == 0),
                                stop=(pc == CH - 1),
                            )
                    if c == 2 and norm_pend is not None:
                        emit_normalize(*norm_pend)
                        norm_pend = None
                    pend = (p, c, et2, utA, utB)
                norm_pend = (p, utA, utB)
            pp_, pc, pets, puA, puB = pend
            for j, ut in enumerate((puA, puB)):
                nc.tensor.matmul(
                    ut[:65, :],
                    vf[:, pc, pp_, ds(j * 65, 65)],
                    pets[:, j, :],
                    start=False,
                    stop=True,
                )
            emit_normalize(*norm_pend, fast=True)

            # ---- output projection + residual(+bo) + LayerNorm ----
            for i in range(SQ):
                res_t = outp.tile([P, D], FP32, tag="res")
                nc.gpsimd.dma_start(res_t[:], res_d[ts(i, P), :])
                po = ps.tile([P, 2, SL], FP32, tag="mm2", bufs=2)
                for n in range(2):
                    for pp2 in range(PAIRS):
                        nc.tensor.matmul(
                            po[:, n, :],
                            ctx[:, pp2, ts(i, P)],
                            wo_sb[:, pp2, ds(n * 512, 512)],
                            start=(pp2 == 0),
                            stop=(pp2 == PAIRS - 1),
                        )
                orow = outp.tile([P, D], FP32, tag="orow")
                nc.vector.tensor_tensor(
                    orow[:], po[:].rearrange("q a s -> q (a s)"), res_t[:], ALU.add
                )
                stats = small.tile([P, 2, 6], FP32, tag="stats")
                nc.vector.bn_stats(stats[:, 0, :], orow[:, 0:512])
                nc.vector.bn_stats(stats[:, 1, :], orow[:, 512:1024])
                mv = small.tile([P, 2], FP32, tag="mv")
                nc.vector.bn_aggr(mv[:], stats[:])
                std = small.tile([P, 1], FP32, tag="std")
                nc.scalar.activation(
                    out=std[:], in_=mv[:, 1:2], func=AF.Sqrt, bias=eps_t[:], scale=1.0
                )
                rstd = small.tile([P, 1], FP32, tag="rstd")
                nc.vector.reciprocal(out=rstd[:], in_=std[:])
                nmr = small.tile([P, 1], FP32, tag="nmr")
                nc.vector.tensor_scalar(
                    out=nmr[:], in0=mv[:, 0:1], scalar1=rstd[:], scalar2=-1.0,
                    op0=ALU.mult, op1=ALU.mult,
                )
                yt = outp.tile([P, D], FP32, tag="yt")
                nc.scalar.activation(
                    out=yt[:], in_=orow[:], func=AF.Identity,
                    bias=nmr[:], scale=rstd[:],
                )
                if apply_gb:
                    nc.vector.tensor_tensor(yt[:], yt[:], gam_b[:], ALU.mult)
                    nc.vector.tensor_tensor(yt[:], yt[:], bet_b[:], ALU.add)
                nc.sync.dma_start(y_d[ts(i, P), :], yt[:])

    nc.compile()
    return nc


def get_nc(apply_gb: bool):
    key = ("nc", apply_gb)
    if key not in _NC_CACHE:
        _NC_CACHE[key] = build_nc(apply_gb)
    return _NC_CACHE[key]


def kernel(
    query,
    key,
    value,
    Wq,
    bq,
    Wk,
    bk,
    Wv,
    bv,
    Wo,
    bo,
    ln_gamma,
    ln_beta,
    _trace=False,
    _trace_cores=None,
):
    query = np.ascontiguousarray(np.asarray(query, dtype=np.float32))
    key = np.ascontiguousarray(np.asarray(key, dtype=np.float32))
    value = np.ascontiguousarray(np.asarray(value, dtype=np.float32))
    bo_f = np.asarray(bo, np.float32)
    gam_f = np.ascontiguousarray(np.asarray(ln_gamma, np.float32))
    bet_f = np.ascontiguousarray(np.asarray(ln_beta, np.float32))
    apply_gb = not (
        np.all(gam_f == np.float32(1.0)) and np.all(bet_f == np.float32(0.0))
    )
    shared = {
        "wq": np.ascontiguousarray(np.asarray(Wq, np.float32).astype(FP8NP)),
        "wk": np.ascontiguousarray(np.asarray(Wk, np.float32).astype(FP8NP)),
        "wv": np.ascontiguousarray(np.asarray(Wv, np.float32).astype(FP8NP)),
        "wo": np.ascontiguousarray(np.asarray(Wo, np.float32).astype(BFNP)),
        "bq": np.ascontiguousarray(np.asarray(bq, np.float32)),
        "bk": np.ascontiguousarray(np.asarray(bk, np.float32)),
        "bv": np.ascontiguousarray(np.asarray(bv, np.float32)),
        "gam": gam_f,
        "bet": bet_f,
        "ones": np.ones((P, 64), dtype=np.float32),
    }
    kT = [np.ascontiguousarray(key[b].T.astype(FP8NP)) for b in range(B)]
    vT = [np.ascontiguousarray(value[b].T.astype(FP8NP)) for b in range(B)]
    in_maps = []
    for c in range(N_CORES):
        b, r = divmod(c, NB)
        rows = slice(r * SL, (r + 1) * SL)
        xq_rows = query[b, rows, :]
        m = dict(shared)
        m["xqt"] = np.ascontiguousarray(xq_rows.T.astype(FP8NP))
        m["xkt"] = kT[b]
        m["xvt"] = vT[b]
        m["resg"] = np.ascontiguousarray(xq_rows + bo_f[None, :])
        in_maps.append(m)

    nc = get_nc(apply_gb)
    res = run_bass_kernel_spmd(
        nc,
        in_maps,
        list(range(N_CORES)),
        trace=_trace,
        trace_cores=_trace_cores,
    )
    out = np.empty((B, S, D), dtype=np.float32)
    for c in range(N_CORES):
        b, r = divmod(c, NB)
        out[b, r * SL : (r + 1) * SL, :] = res.results[c]["y"]
    if _trace:
        return out, res
    return out


# revision 36
# speedup vs baseline: 1.9065x; 1.0147x over previous
"""Multi-head attention + residual + LayerNorm on 8 Trainium2 NeuronCores.

Sharding: core c in 0..7 handles batch b = c//4 and query-row quarter
r = c%4 (rows 512r..512r+512 of S=2048), with ALL 16 heads.  key/value
are replicated per batch (host-side staging); each core computes the
full-sequence K^T and V projections itself — measured collectives on
this stack cost ~130us per 2MB AllGather, far more than the redundant
PE work, and the local pipeline keeps the PE clock warm.

v5 = 288us (vs 542us fp32r baseline; v2 bf16 408us; v3 fused-exp 347us;
v4 fp8-everywhere 330us — fp8 U matmuls reverted: they starved the PE
into HAM cold-clock, making the exp-bound attention phase slower):
  - host stages x^T (pre-transposed); x and Wq/Wk/Wv in fp8e4, the
    rest bf16: no PE transposes, FWL weight loads, 4x fewer DMA bytes
  - QKV projections run fp8 DoubleRow matmuls (2 adjacent k-chunk
    slices per instruction, 2x rate); attention stays bf16
  - K^T, V_aug, Q^T, ctx all SBUF-resident (no DRAM roundtrips)
  - all PSUM matmul tiles are [128, 1024] 2-bank tiles (tag mm2 ring 2
    + softmax accumulators ut ring 4 = 8 banks): projections pair two
    512-col accumulators per tile and evacuate with ONE wide ACT op;
    attention computes both heads' scores into one tile and exps them
    with ONE 1024-wide ACTIVATE (the ACT 352-cycle/instr overhead was
    pacing the attention phase at 1440ns/chunk vs PE's ~1000ns)
  - softmax denominator broadcast via DRAM-bounce DMA (partition-
    stride-0 read) instead of a PE matmul: normalize is entirely off
    the PE critical path, so pairs pipeline without stalls
  - attention software pipeline crosses pair boundaries (U^T matmuls
    of chunk c issue during chunk c+1, last chunk drains into the next
    pair's first chunk)
  - bulk DMAs ride separate engine queues so the Q-projection feed is
    not queued behind the 4MB K/V loads
  - ~36 warmup matmuls + a dummy exp during the DMA preamble warm the
    PE clock (HAM) and preload the ACT exp table
  - LayerNorm: residual+bo folded on host, normalization via one ACT
    op with per-partition scale/bias; gamma/beta applied only when
    they are non-trivial (checked on host, separate compiled variant)

Accumulations stay fp32 in PSUM; softmax reciprocal / LN stats fp32.
"""

import sys

if "/opt/trn_rl_repo" not in sys.path:
    sys.path.insert(0, "/opt/trn_rl_repo")

import ml_dtypes
import numpy as np

import concourse.bacc as bacc
import concourse.bass as bass
import concourse.mybir as mybir
import concourse.tile as tile
from concourse.bass import ds, ts
from concourse.bass_utils import run_bass_kernel_spmd

FP32R = mybir.dt.float32r
FP32 = mybir.dt.float32
BF16 = mybir.dt.bfloat16
FP8 = mybir.dt.float8e4
AF = mybir.ActivationFunctionType
ALU = mybir.AluOpType
DR = mybir.MatmulPerfMode.DoubleRow
BFNP = ml_dtypes.bfloat16
FP8NP = ml_dtypes.float8_e4m3
# exp(s/8 - EXP_SHIFT) keeps softmax weights inside fp8e4 range (max 240);
# the constant shift cancels exactly in the normalize ratio.
EXP_SHIFT = -3.0

N_CORES = 8
B = 2
S = 2048
D = 1024
H = 16
DK = 64
P = 128

SL = S // 4  # 512 local query rows per core
KC = D // P  # 8 contraction chunks over d_model
SQ = SL // P  # 4 sq subchunks of 128 (per 512-row block)
CH = S // P  # 16 sk chunks
PAIRS = H // 2  # 8 head pairs
NB = 4  # row quarters
EPS = 1e-5

_NC_CACHE = {}


def build_nc(apply_gb: bool):
    nc = bacc.Bacc(num_devices=N_CORES)

    xqt_d = nc.dram_tensor("xqt", [D, SL], FP8, kind="ExternalInput")
    xkt_d = nc.dram_tensor("xkt", [D, S], FP8, kind="ExternalInput")
    xvt_d = nc.dram_tensor("xvt", [D, S], FP8, kind="ExternalInput")
    res_d = nc.dram_tensor("resg", [SL, D], FP32, kind="ExternalInput")
    wq_d = nc.dram_tensor("wq", [D, D], FP8, kind="ExternalInput")
    wk_d = nc.dram_tensor("wk", [D, D], FP8, kind="ExternalInput")
    wv_d = nc.dram_tensor("wv", [D, D], FP8, kind="ExternalInput")
    wo_d = nc.dram_tensor("wo", [D, D], BF16, kind="ExternalInput")
    bq_d = nc.dram_tensor("bq", [D], FP32, kind="ExternalInput")
    bk_d = nc.dram_tensor("bk", [D], FP32, kind="ExternalInput")
    bv_d = nc.dram_tensor("bv", [D], FP32, kind="ExternalInput")
    gam_d = nc.dram_tensor("gam", [D], FP32, kind="ExternalInput")
    bet_d = nc.dram_tensor("bet", [D], FP32, kind="ExternalInput")

    ones_d = nc.dram_tensor("ones", [P, 64], FP32R, kind="ExternalInput")
    y_d = nc.dram_tensor("y", [SL, D], FP32, kind="ExternalOutput")
    # scratch for the softmax-denominator partition broadcast
    zsc_d = nc.dram_tensor("zsc", [PAIRS * 2 * SL], FP32R)

    wq_r = wq_d.rearrange("(c q) m -> q c m", q=P)
    wk_r = wk_d.rearrange("(c q) m -> q c m", q=P)
    wv_r = wv_d.rearrange("(c q) m -> q c m", q=P)
    wo_r = wo_d.rearrange("(c q) m -> q c m", q=P)
    xqt_r = xqt_d.rearrange("(c q) s -> q c s", q=P)
    xkt_r = xkt_d.rearrange("(c q) s -> q c s", q=P)
    xvt_r = xvt_d.rearrange("(c q) s -> q c s", q=P)

    with tile.TileContext(nc) as tc:
        with (
            tc.tile_pool(name="consts", bufs=1) as consts,
            tc.tile_pool(name="big", bufs=1) as big,
            tc.tile_pool(name="xvp", bufs=3) as xvp,
            tc.tile_pool(name="wcol", bufs=3) as wcol,
            tc.tile_pool(name="wres", bufs=1) as wres,
            tc.tile_pool(name="etp", bufs=3) as etp,
            tc.tile_pool(name="normp", bufs=2) as normp,
            tc.tile_pool(name="outp", bufs=2) as outp,
            tc.tile_pool(name="small", bufs=2) as small,
            tc.tile_pool(name="ps", bufs=1, space="PSUM") as ps,
        ):
            # ---- constants + early DMAs for the Q projection ----
            bq_sb = consts.tile([P, KC], FP32)
            nc.sync.dma_start(bq_sb[:], bq_d.rearrange("(m q) -> q m", q=P))
            bk_sb = consts.tile([P, KC], FP32)
            nc.sync.dma_start(bk_sb[:], bk_d.rearrange("(m q) -> q m", q=P))
            xqT = big.tile([P, KC, SL], FP8, tag="xqT")
            nc.sync.dma_start(xqT[:, 0:4, :], xqt_r[:, 0:4, :])
            nc.gpsimd.dma_start(xqT[:, 4:8, :], xqt_r[:, 4:8, :])
            # bulk loads split across queues so they stream in parallel;
            # nothing else may sit in front of these on gpsimd — the
            # broadcast-bias loads go after the K projection instead
            xkT = big.tile([P, KC, S], FP8, tag="xkT")
            nc.scalar.dma_start(xkT[:, 0:4, :], xkt_r[:, 0:4, :])
            nc.gpsimd.dma_start(xkT[:, 4:8, :], xkt_r[:, 4:8, :])
            wv_sb = wres.tile([P, KC, D], FP8, tag="wres8")
            nc.scalar.dma_start(wv_sb[:], wv_r[:])

            def bcast_load(src, tag, dt):
                t = consts.tile([P, D], dt, tag=tag)
                ap = bass.AP(tensor=src, offset=0, ap=[[0, P], [1, D]])
                nc.gpsimd.dma_start(out=t[:], in_=ap)
                return t
            eps_t = consts.tile([P, 1], FP32)
            nc.vector.memset(eps_t[:], EPS)
            on64 = consts.tile([P, 64], FP32R)
            nc.sync.dma_start(on64[:], ones_d[:])

            # ---- PE clock warmup + ACT exp table preload (runs during
            # the input DMA preamble; results are never read) ----
            warm = consts.tile([P, P], BF16)
            nc.vector.memset(warm[:], 0.001)
            wx1 = consts.tile([P, 1], FP32)
            nc.vector.memset(wx1[:], 0.0)
            wxo = consts.tile([P, 1], BF16)
            nc.scalar.activation(out=wxo[:], in_=wx1[:], func=AF.Exp, scale=0.125)
            for _ in range(52):
                pw = ps.tile([P, 2, SL], FP32, tag="mm2", bufs=2)
                nc.tensor.matmul(pw[:, 0, 0:P], warm[:], warm[:], start=True, stop=True)

            # ---- Q^T projection (own rows): qt[q, m, s] = Q^T ----
            qt = big.tile([P, KC, SL], BF16, tag="qt")
            for m in range(KC):
                wq_t = wcol.tile([P, KC, P], FP8, tag="wcol")
                nc.sync.dma_start(wq_t[:], wq_r[:, :, ts(m, P)])
                pq = ps.tile([P, 2, SL], FP32, tag="mm2", bufs=2)
                for k in range(0, KC, 2):
                    nc.tensor.matmul(
                        pq[:, 0, :],
                        wq_t[:, k : k + 2, :],
                        xqT[:, k : k + 2, :],
                        start=(k == 0),
                        stop=(k == KC - 2),
                        perf_mode=DR,
                    )
                nc.scalar.activation(
                    out=qt[:, m, :], in_=pq[:, 0, :], func=AF.Identity,
                    bias=bq_sb[:, m : m + 1],
                )

            # ---- K^T projection (full sequence, SBUF-resident) ----
            ktf = big.tile([P, KC, S], BF16, tag="ktf")
            for m in range(KC):
                wk_t = wcol.tile([P, KC, P], FP8, tag="wcol")
                nc.sync.dma_start(wk_t[:], wk_r[:, :, ts(m, P)])
                for g in range(2):
                    pk = ps.tile([P, 2, SL], FP32, tag="mm2", bufs=2)
                    for k in range(0, KC, 2):
                        for q2 in range(2):
                            nc.tensor.matmul(
                                pk[:, q2, :],
                                wk_t[:, k : k + 2, :],
                                xkT[:, k : k + 2, ds((g * 2 + q2) * SL, SL)],
                                start=(k == 0),
                                stop=(k == KC - 2),
                                perf_mode=DR,
                            )
                    # alternate evacuation between ACT and DVE so neither
                    # engine paces the fp8 matmul stream
                    if g == 0:
                        nc.scalar.activation(
                            out=ktf[:, m, ds(g * 1024, 1024)],
                            in_=pk[:].rearrange("q a s -> q (a s)"),
                            func=AF.Identity,
                            bias=bk_sb[:, m : m + 1],
                        )
                    else:
                        nc.vector.tensor_scalar_add(
                            ktf[:, m, ds(g * 1024, 1024)],
                            pk[:].rearrange("q a s -> q (a s)"),
                            bk_sb[:, m : m + 1],
                        )

            # broadcast-bias loads ride gpsimd after the critical preamble
            bv_b = bcast_load(bv_d, "bv_b", FP32)
            if apply_gb:
                gam_b = bcast_load(gam_d, "gam_b", FP32)
                bet_b = bcast_load(bet_d, "bet_b", FP32)

            # ---- V projection (full sequence, pair-augmented, SBUF) ----
            # vf[q, sc, pair, 130] = [V_even 64 | 1 | V_odd 64 | 1] bf16
            vf = big.tile([P, CH, PAIRS, 130], BF16, tag="vf")
            nc.vector.memset(vf[:, :, :, 64:65], 1.0)
            nc.vector.memset(vf[:, :, :, 129:130], 1.0)
            for sc in range(CH):
                xv_t = xvp.tile([P, KC, P], FP8, tag="xv")
                nc.sync.dma_start(xv_t[:], xvt_r[:, :, ts(sc, P)])
                pv = ps.tile([P, 2, SL], FP32, tag="mm2", bufs=2)
                for k in range(0, KC, 2):
                    for half in range(2):
                        nc.tensor.matmul(
                            pv[:, half, :],
                            xv_t[:, k : k + 2, :],
                            wv_sb[:, k : k + 2, ds(half * 512, 512)],
                            start=(k == 0),
                            stop=(k == KC - 2),
                            perf_mode=DR,
                        )
                for half in range(2):
                    vdst = vf[:, sc, ds(half * 4, 4), :].rearrange(
                        "q pl (j e) -> q pl j e", e=65
                    )
                    nc.vector.tensor_tensor(
                        vdst[:, :, :, 0:64],
                        pv[:, half, :].rearrange("q (pl j e) -> q pl j e", pl=4, j=2),
                        bv_b[:, ds(half * 512, 512)].rearrange(
                            "q (pl j e) -> q pl j e", pl=4, j=2
                        ),
                        ALU.add,
                    )

            # wo and the residual rows load during attention on idle queues
            wo_sb = wres.tile([P, KC, D], BF16, tag="wres")
            nc.scalar.dma_start(wo_sb[:], wo_r[:])
            res_ts = []
            for i in range(SQ):
                res_t = outp.tile([P, D], FP32, tag="res", bufs=4, name=f"res_{i}")
                nc.gpsimd.dma_start(res_t[:], res_d[ts(i, P), :])
                res_ts.append(res_t)

            # ---- attention ----
            ctx = big.tile([P, PAIRS, SL], BF16, tag="ctx")

            def emit_normalize(p_, uA, uB, fast=False):
                # rows 0..63 of ut / row 64 -> ctx[:, p_, :].  The
                # denominator reciprocal is broadcast to 64 partitions by
                # a DRAM bounce (stride-0 partition read) — no PE, no PSUM.
                # The last pair uses a PE broadcast instead: the bounce
                # latency (~12us) would gate the output projection.
                for j, ut in enumerate((uA, uB)):
                    rec = normp.tile([P, SL], FP32R, tag="rec")
                    with nc.allow_low_precision(
                        reason="float32r is bit-identical to float32 in SBUF"
                    ):
                        nc.vector.reciprocal(out=rec[64:65, :], in_=ut[64:65, :])
                    bcs = normp.tile([P, SL], FP32R, tag="bcs")
                    if fast:
                        bcp = ps.tile([P, 2, SL], FP32, tag="mm2", bufs=2)
                        nc.tensor.matmul(
                            bcp[0:64, 0, :],
                            on64[64:65, :],
                            rec[64:65, :],
                            start=True,
                            stop=True,
                        )
                        nc.vector.tensor_copy(bcs[0:64, :], bcp[0:64, 0, :])
                    else:
                        off = (p_ * 2 + j) * SL
                        nc.sync.dma_start(
                            bass.AP(tensor=zsc_d, offset=off, ap=[[0, 1], [1, SL]]),
                            rec[64:65, :],
                        )
                        nc.sync.dma_start(
                            bcs[0:64, :],
                            bass.AP(tensor=zsc_d, offset=off, ap=[[0, 64], [1, SL]]),
                        )
                    if j == 0:
                        nc.vector.tensor_tensor(
                            ctx[0:64, p_, :], ut[0:64, :], bcs[0:64, :], ALU.mult
                        )
                    else:
                        ctmp = normp.tile([P, SL], BF16, tag="ctmp")
                        nc.vector.tensor_tensor(
                            ctmp[0:64, :], ut[0:64, :], bcs[0:64, :], ALU.mult
                        )
                        # partition shift 0-63 -> 64-127 via SBUF-SBUF DMA
                        nc.gpsimd.dma_start(ctx[64:128, p_, :], ctmp[0:64, :])

            pend = None
            norm_pend = None
            for p in range(PAIRS):
                utA = ps.tile([P, SL], FP32, tag="ut", bufs=4)
                utB = ps.tile([P, SL], FP32, tag="ut", bufs=4)
                for c in range(CH):
                    st2 = ps.tile([P, 2, SL], FP32, tag="mm2", bufs=2)
                    for j in range(2):
                        nc.tensor.matmul(
                            st2[:, j, :],
                            ktf[ds(j * 64, 64), p, ts(c, P)],
                            qt[ds(j * 64, 64), p, :],
                            start=True,
                            stop=True,
                        )
                    et2 = etp.tile([P, 2, SL], BF16, tag="et")
                    nc.scalar.activation(
                        out=et2[:], in_=st2[:], func=AF.Exp, scale=0.125
                    )
                    if pend is not None:
                        pp_, pc, pets, puA, puB = pend
                        for j, ut in enumerate((puA, puB)):
                            nc.tensor.matmul(
                                ut[:65, :],
                                vf[:, pc, pp_, ds(j * 65, 65)],
                                pets[:, j, :],
                                start=(pc == 0),
                                stop=(pc == CH - 1),
                            )
                    if c == 2 and norm_pend is not None:
                        emit_normalize(*norm_pend)
                        norm_pend = None
                    pend = (p, c, et2, utA, utB)
                norm_pend = (p, utA, utB)
            pp_, pc, pets, puA, puB = pend
            for j, ut in enumerate((puA, puB)):
                nc.tensor.matmul(
                    ut[:65, :],
                    vf[:, pc, pp_, ds(j * 65, 65)],
                    pets[:, j, :],
                    start=False,
                    stop=True,
                )
            emit_normalize(*norm_pend, fast=True)

            # ---- output projection + residual(+bo) + LayerNorm ----
            for i in range(SQ):
                res_t = res_ts[i]
                po = ps.tile([P, 2, SL], FP32, tag="mm2", bufs=2)
                for n in range(2):
                    for pp2 in range(PAIRS):
                        nc.tensor.matmul(
                            po[:, n, :],
                            ctx[:, pp2, ts(i, P)],
                            wo_sb[:, pp2, ds(n * 512, 512)],
                            start=(pp2 == 0),
                            stop=(pp2 == PAIRS - 1),
                        )
                orow = outp.tile([P, D], FP32, tag="orow")
                nc.vector.tensor_tensor(
                    orow[:], po[:].rearrange("q a s -> q (a s)"), res_t[:], ALU.add
                )
                stats = small.tile([P, 2, 6], FP32, tag="stats")
                nc.vector.bn_stats(stats[:, 0, :], orow[:, 0:512])
                nc.vector.bn_stats(stats[:, 1, :], orow[:, 512:1024])
                mv = small.tile([P, 2], FP32, tag="mv")
                nc.vector.bn_aggr(mv[:], stats[:])
                std = small.tile([P, 1], FP32, tag="std")
                nc.scalar.activation(
                    out=std[:], in_=mv[:, 1:2], func=AF.Sqrt, bias=eps_t[:], scale=1.0
                )
                rstd = small.tile([P, 1], FP32, tag="rstd")
                nc.vector.reciprocal(out=rstd[:], in_=std[:])
                nmr = small.tile([P, 1], FP32, tag="nmr")
                nc.vector.tensor_scalar(
                    out=nmr[:], in0=mv[:, 0:1], scalar1=rstd[:], scalar2=-1.0,
                    op0=ALU.mult, op1=ALU.mult,
                )
                yt = outp.tile([P, D], FP32, tag="yt")
                nc.scalar.activation(
                    out=yt[:], in_=orow[:], func=AF.Identity,
                    bias=nmr[:], scale=rstd[:],
                )
                if apply_gb:
                    nc.vector.tensor_tensor(yt[:], yt[:], gam_b[:], ALU.mult)
                    nc.vector.tensor_tensor(yt[:], yt[:], bet_b[:], ALU.add)
                # split the store across two queues to halve the drain
                nc.sync.dma_start(y_d[ts(i, P), 0:512], yt[:, 0:512])
                nc.scalar.dma_start(y_d[ts(i, P), 512:1024], yt[:, 512:1024])

    nc.compile()
    return nc


def get_nc(apply_gb: bool):
    key = ("nc", apply_gb)
    if key not in _NC_CACHE:
        _NC_CACHE[key] = build_nc(apply_gb)
    return _NC_CACHE[key]


def kernel(
    query,
    key,
    value,
    Wq,
    bq,
    Wk,
    bk,
    Wv,
    bv,
    Wo,
    bo,
    ln_gamma,
    ln_beta,
    _trace=False,
    _trace_cores=None,
):
    query = np.ascontiguousarray(np.asarray(query, dtype=np.float32))
    key = np.ascontiguousarray(np.asarray(key, dtype=np.float32))
    value = np.ascontiguousarray(np.asarray(value, dtype=np.float32))
    bo_f = np.asarray(bo, np.float32)
    gam_f = np.ascontiguousarray(np.asarray(ln_gamma, np.float32))
    bet_f = np.ascontiguousarray(np.asarray(ln_beta, np.float32))
    apply_gb = not (
        np.all(gam_f == np.float32(1.0)) and np.all(bet_f == np.float32(0.0))
    )
    shared = {
        "wq": np.ascontiguousarray(np.asarray(Wq, np.float32).astype(FP8NP)),
        "wk": np.ascontiguousarray(np.asarray(Wk, np.float32).astype(FP8NP)),
        "wv": np.ascontiguousarray(np.asarray(Wv, np.float32).astype(FP8NP)),
        "wo": np.ascontiguousarray(np.asarray(Wo, np.float32).astype(BFNP)),
        "bq": np.ascontiguousarray(np.asarray(bq, np.float32)),
        "bk": np.ascontiguousarray(np.asarray(bk, np.float32)),
        "bv": np.ascontiguousarray(np.asarray(bv, np.float32)),
        "gam": gam_f,
        "bet": bet_f,
        "ones": np.ones((P, 64), dtype=np.float32),
    }
    kT = [np.ascontiguousarray(key[b].T.astype(FP8NP)) for b in range(B)]
    vT = [np.ascontiguousarray(value[b].T.astype(FP8NP)) for b in range(B)]
    in_maps = []
    for c in range(N_CORES):
        b, r = divmod(c, NB)
        rows = slice(r * SL, (r + 1) * SL)
        xq_rows = query[b, rows, :]
        m = dict(shared)
        m["xqt"] = np.ascontiguousarray(xq_rows.T.astype(FP8NP))
        m["xkt"] = kT[b]
        m["xvt"] = vT[b]
        m["resg"] = np.ascontiguousarray(xq_rows + bo_f[None, :])
        in_maps.append(m)

    nc = get_nc(apply_gb)
    res = run_bass_kernel_spmd(
        nc,
        in_maps,
        list(range(N_CORES)),
        trace=_trace,
        trace_cores=_trace_cores,
    )
    out = np.empty((B, S, D), dtype=np.float32)
    for c in range(N_CORES):
        b, r = divmod(c, NB)
        out[b, r * SL : (r + 1) * SL, :] = res.results[c]["y"]
    if _trace:
        return out, res
    return out
